# revision 1
# baseline (speedup 1.0000x reference)
"""Trainium2 Bass kernel for nn_EqPropTuned (equilibrium-propagation relaxation).

Network: DIMS = [2048, 2048, 2048, 2048, 1000], BATCH = 1024, 25 Gauss-Seidel
sweeps with lr 0.3, rho = clip(0, 1).

Sharding: data-parallel over batch across 8 cores (128 rows/core), weights
replicated. All states kept on-chip in dim-major ("transposed") layout
[dim, batch_per_core]; weight matrices streamed from HBM per sweep as
pre-tiled fp16 slabs (forward and pre-transposed backward copies). Matmuls
run in fp16 (fp32 PSUM accumulate); the master states stay fp32 on-chip,
with fp16 mirror copies feeding the PE.

Key algebraic facts used:
  - states are clipped in place, so rho() on a stored state is the identity
  - rho(x) @ W0 + b1 is constant across sweeps -> computed once at init (c1)
  - forward + backward matmul terms for one state tile accumulate into one
    PSUM group
"""

import os
import numpy as np
from contextlib import ExitStack

import concourse.bass as bass
import concourse.tile as tile
from concourse import mybir, bacc
from concourse.bass_utils import run_bass_kernel_spmd

F32 = mybir.dt.float32
F16 = mybir.dt.float16
AL = mybir.AluOpType

P = 128
DIMS = [2048, 2048, 2048, 2048, 1000]
PD = [2048, 2048, 2048, 2048, 1024]  # padded dims
KT = [d // P for d in PD]            # [16, 16, 16, 16, 8] k-tiles per dim
BATCH = 1024
N_CORES = 8
BPC = BATCH // N_CORES               # 128 batch rows per core
N_RELAX = int(os.environ.get("KERNEL_N_RELAX", "25"))
LR = 0.3

MM_DT = F16
MM_NP = np.float16


def _slab_f(W, Kp, Mp):
    """Forward slabs: out[m, p, k*P+j] = W[k*P+p, m*P+j], shape [Mp/P, P, Kp]."""
    K, M = W.shape
    Wp = np.zeros((Kp, Mp), np.float32)
    Wp[:K, :M] = W
    t = Wp.reshape(Kp // P, P, Mp // P, P)  # [k, p, m, j]
    out = np.ascontiguousarray(t.transpose(2, 1, 0, 3)).reshape(Mp // P, P, Kp)
    return out.astype(MM_NP)


def _slab_b(W, Kp, Mp):
    """Backward slabs built from W.T (contract over W's output dim)."""
    return _slab_f(np.ascontiguousarray(W.T.astype(np.float32)), Kp, Mp)


def _bias_tiles(b, Mp, scale=1.0):
    """[P, Mp/P] with out[p, m] = scale * b[m*P+p]."""
    bp = np.zeros(Mp, np.float32)
    bp[: b.shape[0]] = b * scale
    return np.ascontiguousarray(bp.reshape(Mp // P, P).T)


def build_nc():
    nc = bacc.Bacc(None, target_bir_lowering=False, debug=False)

    d_x16 = nc.declare_dram_parameter("x16T", [P, PD[0]], F16, isOutput=False)
    d_cx16 = nc.declare_dram_parameter("cx16T", [P, PD[0]], F16, isOutput=False)
    d_w = {}
    # forward slabs for W0..W3: contract over DIMS[l], output DIMS[l+1]
    for l in range(4):
        d_w[f"w{l}f"] = nc.declare_dram_parameter(
            f"w{l}f", [PD[l + 1] // P, P, PD[l]], MM_DT, isOutput=False
        )
    # backward slabs for W1..W3: contract over DIMS[l+1], output DIMS[l]
    for l in range(1, 4):
        d_w[f"w{l}b"] = nc.declare_dram_parameter(
            f"w{l}b", [PD[l] // P, P, PD[l + 1]], MM_DT, isOutput=False
        )
    d_b = {}
    for l in range(1, 5):
        d_b[f"b{l}raw"] = nc.declare_dram_parameter(
            f"b{l}raw", [P, PD[l] // P], F32, isOutput=False
        )
        d_b[f"b{l}s"] = nc.declare_dram_parameter(
            f"b{l}s", [P, PD[l] // P], F32, isOutput=False
        )
    d_out = nc.declare_dram_parameter("out", [P, PD[4]], F32, isOutput=True)

    with tile.TileContext(nc) as tc, ExitStack() as ctx:
        st = ctx.enter_context(tc.tile_pool(name="state", bufs=1))
        wp = ctx.enter_context(tc.tile_pool(name="wslab", bufs=4))
        pp = ctx.enter_context(tc.tile_pool(name="psum", bufs=8, space="PSUM"))
        tp = ctx.enter_context(tc.tile_pool(name="tmp", bufs=6))

        # persistent tensors: fp32 master states + fp16 matmul mirrors
        s = {}
        s16 = {}
        for l in range(1, 5):
            s[l] = st.tile([P, PD[l]], F32, tag=f"s{l}", name=f"s{l}")
            s16[l] = st.tile([P, PD[l]], MM_DT, tag=f"s16_{l}", name=f"s16_{l}")
        c1s = st.tile([P, PD[1]], F16, tag="c1s")
        x16 = st.tile([P, PD[0]], MM_DT, tag="x16")
        cx16 = st.tile([P, PD[0]], MM_DT, tag="cx16")
        bias = {}
        for l in range(1, 5):
            bias[f"b{l}raw"] = st.tile(
                [P, PD[l] // P], F32, tag=f"b{l}raw", name=f"b{l}raw"
            )
            bias[f"b{l}s"] = st.tile(
                [P, PD[l] // P], F32, tag=f"b{l}s", name=f"b{l}s"
            )
            nc.sync.dma_start(bias[f"b{l}raw"][:], d_b[f"b{l}raw"][:])
            nc.sync.dma_start(bias[f"b{l}s"][:], d_b[f"b{l}s"][:])

        nc.sync.dma_start(x16[:], d_x16[:])
        nc.sync.dma_start(cx16[:], d_cx16[:])

        def mm_group(psum, slab, rhs16, kt, first, last):
            for k in range(kt):
                nc.tensor.matmul(
                    psum[:],
                    slab[:, bass.ts(k, P)],
                    rhs16[:, bass.ts(k, P)],
                    start=(first and k == 0),
                    stop=(last and k == kt - 1),
                )

        # ---- init pass ----
        # layer 1 init + c1 constant share one pass over w0f
        for m in range(KT[1]):
            wf = wp.tile([P, PD[0]], MM_DT, tag="slab")
            nc.sync.dma_start(wf[:], d_w["w0f"][m])
            ps_i = pp.tile([P, P], F32, tag="ps")
            ps_c = pp.tile([P, P], F32, tag="ps")
            mm_group(ps_i, wf, x16, KT[0], True, True)
            mm_group(ps_c, wf, cx16, KT[0], True, True)
            # s1_init = clip(x @ W0 + b1)
            t = tp.tile([P, P], F32, tag="t")
            nc.vector.tensor_scalar(
                t[:], ps_i[:], bias["b1raw"][:, m : m + 1], 0.0, AL.add, AL.max
            )
            nc.vector.tensor_scalar_min(s[1][:, bass.ts(m, P)], t[:], 1.0)
            nc.gpsimd.tensor_scalar_min(s16[1][:, bass.ts(m, P)], t[:], 1.0)
            # c1s = 0.3 * (clip(x) @ W0 + b1)
            nc.vector.tensor_scalar(
                c1s[:, bass.ts(m, P)],
                ps_c[:],
                0.3,
                bias["b1s"][:, m : m + 1],
                AL.mult,
                AL.add,
            )

        # W3 (smallest matrix) stays resident in SBUF for all sweeps:
        # saves 8 MB/sweep of HBM streaming.
        w3f_res = st.tile([P, KT[4] * PD[3]], MM_DT, tag="w3f_res")
        w3b_res = st.tile([P, KT[3] * PD[4]], MM_DT, tag="w3b_res")
        for m in range(KT[4]):
            nc.sync.dma_start(
                w3f_res[:, m * PD[3] : (m + 1) * PD[3]], d_w["w3f"][m]
            )
        for m in range(KT[3]):
            nc.sync.dma_start(
                w3b_res[:, m * PD[4] : (m + 1) * PD[4]], d_w["w3b"][m]
            )
        # partial residency for W2 backward slabs (as many as SBUF allows)
        N_W2B_RES = 12
        w2b_res = st.tile([P, N_W2B_RES * PD[3]], MM_DT, tag="w2b_res")
        for m in range(N_W2B_RES):
            nc.sync.dma_start(
                w2b_res[:, m * PD[3] : (m + 1) * PD[3]], d_w["w2b"][m]
            )

        # init layers 2..4: s_{l+1} = clip(s_l @ W_l + b_{l+1})
        for l in range(1, 4):
            for m in range(KT[l + 1]):
                if l == 3:
                    wf = w3f_res[:, m * PD[3] : (m + 1) * PD[3]]
                else:
                    wf = wp.tile([P, PD[l]], MM_DT, tag="slab")
                    nc.sync.dma_start(wf[:], d_w[f"w{l}f"][m])
                ps = pp.tile([P, P], F32, tag="ps")
                mm_group(ps, wf, s16[l], KT[l], True, True)
                t = tp.tile([P, P], F32, tag="t")
                nc.vector.tensor_scalar(
                    t[:],
                    ps[:],
                    bias[f"b{l + 1}raw"][:, m : m + 1],
                    0.0,
                    AL.add,
                    AL.max,
                )
                nc.vector.tensor_scalar_min(s[l + 1][:, bass.ts(m, P)], t[:], 1.0)
                nc.gpsimd.tensor_scalar_min(
                    s16[l + 1][:, bass.ts(m, P)], t[:], 1.0
                )

        # ---- relaxation sweeps ----
        # streamed slabs are fetched in adjacent-m pairs (one 1 MB DMA instead
        # of two 0.5 MB ones) for better HBM efficiency
        for _ in range(N_RELAX):
            for l in range(1, 5):
                fwd = None if l == 1 else (d_w[f"w{l - 1}f"], s16[l - 1], KT[l - 1])
                bwd = None if l == 4 else (d_w[f"w{l}b"], s16[l + 1], KT[l + 1])
                pair_f = pair_b = None
                for m in range(KT[l]):
                    if m % 2 == 0:
                        pair_f = pair_b = None
                        if fwd is not None and l != 4:
                            kf = fwd[2] * P
                            pair_f = wp.tile([P, 2 * kf], MM_DT, tag="slab")
                            nc.sync.dma_start(
                                pair_f[:].rearrange("p (i k) -> p i k", i=2),
                                fwd[0][m : m + 2].rearrange("i p k -> p i k"),
                            )
                        if bwd is not None and l != 3 and not (
                            l == 2 and m + 1 < N_W2B_RES
                        ):
                            kb = bwd[2] * P
                            pair_b = wp.tile([P, 2 * kb], MM_DT, tag="slab")
                            nc.sync.dma_start(
                                pair_b[:].rearrange("p (i k) -> p i k", i=2),
                                bwd[0][m : m + 2].rearrange("i p k -> p i k"),
                            )
                    slabs = []
                    if fwd is not None:
                        if l == 4:
                            wf = w3f_res[:, m * PD[3] : (m + 1) * PD[3]]
                        else:
                            kf = fwd[2] * P
                            wf = pair_f[:, (m % 2) * kf : (m % 2 + 1) * kf]
                        slabs.append((wf, fwd[1], fwd[2]))
                    if bwd is not None:
                        if l == 3:
                            wb = w3b_res[:, m * PD[4] : (m + 1) * PD[4]]
                        elif l == 2 and m < N_W2B_RES:
                            wb = w2b_res[:, m * PD[3] : (m + 1) * PD[3]]
                        else:
                            kb = bwd[2] * P
                            wb = pair_b[:, (m % 2) * kb : (m % 2 + 1) * kb]
                        slabs.append((wb, bwd[1], bwd[2]))
                    ps = pp.tile([P, P], F32, tag="ps")
                    for i, (slab, rhs16, kt) in enumerate(slabs):
                        mm_group(ps, slab, rhs16, kt, i == 0, i == len(slabs) - 1)
                    # t = 0.3 * psum + 0.3 * bias   (or + 0.3 * c1 for l=1)
                    t = tp.tile([P, P], F32, tag="t")
                    if l == 1:
                        nc.vector.scalar_tensor_tensor(
                            t[:], ps[:], 0.3, c1s[:, bass.ts(m, P)], AL.mult, AL.add
                        )
                    else:
                        nc.vector.tensor_scalar(
                            t[:], ps[:], 0.3, bias[f"b{l}s"][:, m : m + 1],
                            AL.mult, AL.add,
                        )
                    # u = 0.7 * s + t ; s = clip(u, 0, 1) (fp32 + fp16 mirror)
                    u = tp.tile([P, P], F32, tag="u")
                    nc.vector.scalar_tensor_tensor(
                        u[:], s[l][:, bass.ts(m, P)], 0.7, t[:], AL.mult, AL.add
                    )
                    nc.vector.tensor_scalar(
                        s[l][:, bass.ts(m, P)], u[:], 0.0, 1.0, AL.max, AL.min
                    )
                    nc.gpsimd.tensor_scalar(
                        s16[l][:, bass.ts(m, P)], u[:], 0.0, 1.0, AL.max, AL.min
                    )

        nc.sync.dma_start(d_out[:], s[4][:])

    nc.compile()
    return nc


def _prep_inputs(x, W0, W1, W2, W3, b1, b2, b3, b4):
    """Host-side data prep shared by all cores (weights) + per-core x."""
    common = {
        "w0f": _slab_f(W0, PD[0], PD[1]),
        "w1f": _slab_f(W1, PD[1], PD[2]),
        "w2f": _slab_f(W2, PD[2], PD[3]),
        "w3f": _slab_f(W3, PD[3], PD[4]),
        "w1b": _slab_b(W1, PD[2], PD[1]),
        "w2b": _slab_b(W2, PD[3], PD[2]),
        "w3b": _slab_b(W3, PD[4], PD[3]),
    }
    for l, b in zip(range(1, 5), [b1, b2, b3, b4]):
        common[f"b{l}raw"] = _bias_tiles(b, PD[l], 1.0)
        common[f"b{l}s"] = _bias_tiles(b, PD[l], LR)

    in_maps = []
    for c in range(N_CORES):
        xs = np.asarray(x[c * BPC : (c + 1) * BPC], dtype=np.float32)
        # xT[p, k*P+j] = xs[j, k*P+p]
        xT = np.ascontiguousarray(
            xs.reshape(BPC, PD[0] // P, P).transpose(2, 1, 0)
        ).reshape(P, PD[0])
        in_maps.append({
            "x16T": xT.astype(np.float16),
            "cx16T": np.clip(xT, 0.0, 1.0).astype(np.float16),
            **common,
        })
    return in_maps


_NC_CACHE = None


def _get_nc():
    global _NC_CACHE
    if _NC_CACHE is None:
        _NC_CACHE = build_nc()
    return _NC_CACHE


def run(inputs, trace=False):
    nc = _get_nc()
    in_maps = _prep_inputs(**inputs)
    res = run_bass_kernel_spmd(nc, in_maps, list(range(N_CORES)), trace=trace)
    outs = []
    for c in range(N_CORES):
        o = res.results[c]["out"]  # [P, PD[4]] = [128, 1024]
        # decode: o[p, k*P+j] = s4T[k*P+p, j] = s4[batch j, dim k*P+p]
        s4 = o.reshape(P, PD[4] // P, P).transpose(2, 1, 0).reshape(BPC, PD[4])
        outs.append(s4[:, : DIMS[4]])
    return np.concatenate(outs, axis=0).astype(np.float32), res


def kernel(**inputs):
    out, _ = run(inputs, trace=False)
    return out



# revision 2
# speedup vs baseline: 24.9560x; 24.9560x over previous
"""Trainium2 Bass kernel for nn_EqPropTuned (equilibrium-propagation relaxation).

Network: DIMS = [2048, 2048, 2048, 2048, 1000], BATCH = 1024, 25 Gauss-Seidel
sweeps with lr 0.3, rho = clip(0, 1).

Sharding: data-parallel over batch across 8 cores (128 rows/core), weights
replicated. All states kept on-chip in dim-major ("transposed") layout
[dim, batch_per_core]; weight matrices streamed from HBM per sweep as
pre-tiled fp16 slabs (forward and pre-transposed backward copies). Matmuls
run in fp16 (fp32 PSUM accumulate); the master states stay fp32 on-chip,
with fp16 mirror copies feeding the PE.

Key algebraic facts used:
  - states are clipped in place, so rho() on a stored state is the identity
  - rho(x) @ W0 + b1 is constant across sweeps -> computed once at init (c1)
  - forward + backward matmul terms for one state tile accumulate into one
    PSUM group
"""

import os
import numpy as np
from contextlib import ExitStack

import concourse.bass as bass
import concourse.tile as tile
from concourse import mybir, bacc
from concourse.bass_utils import run_bass_kernel_spmd

F32 = mybir.dt.float32
F16 = mybir.dt.float16
AL = mybir.AluOpType

P = 128
DIMS = [2048, 2048, 2048, 2048, 1000]
PD = [2048, 2048, 2048, 2048, 1024]  # padded dims
KT = [d // P for d in PD]            # [16, 16, 16, 16, 8] k-tiles per dim
BATCH = 1024
N_CORES = 8
BPC = BATCH // N_CORES               # 128 batch rows per core
N_RELAX = int(os.environ.get("KERNEL_N_RELAX", "25"))
LR = 0.3

MM_DT = F16
MM_NP = np.float16


def _slab_f(W, Kp, Mp):
    """Forward slabs: out[m, p, k*P+j] = W[k*P+p, m*P+j], shape [Mp/P, P, Kp]."""
    K, M = W.shape
    Wp = np.zeros((Kp, Mp), np.float32)
    Wp[:K, :M] = W
    t = Wp.reshape(Kp // P, P, Mp // P, P)  # [k, p, m, j]
    out = np.ascontiguousarray(t.transpose(2, 1, 0, 3)).reshape(Mp // P, P, Kp)
    return out.astype(MM_NP)


def _slab_b(W, Kp, Mp):
    """Backward slabs built from W.T (contract over W's output dim)."""
    return _slab_f(np.ascontiguousarray(W.T.astype(np.float32)), Kp, Mp)


def _bias_tiles(b, Mp, scale=1.0):
    """[P, Mp/P] with out[p, m] = scale * b[m*P+p]."""
    bp = np.zeros(Mp, np.float32)
    bp[: b.shape[0]] = b * scale
    return np.ascontiguousarray(bp.reshape(Mp // P, P).T)


def build_nc():
    nc = bacc.Bacc(None, target_bir_lowering=False, debug=False)

    d_x16 = nc.declare_dram_parameter("x16T", [P, PD[0]], F16, isOutput=False)
    d_cx16 = nc.declare_dram_parameter("cx16T", [P, PD[0]], F16, isOutput=False)
    d_w = {}
    # forward slabs for W0..W3: contract over DIMS[l], output DIMS[l+1]
    for l in range(4):
        d_w[f"w{l}f"] = nc.declare_dram_parameter(
            f"w{l}f", [PD[l + 1] // P, P, PD[l]], MM_DT, isOutput=False
        )
    # backward slabs for W1..W3: contract over DIMS[l+1], output DIMS[l]
    for l in range(1, 4):
        d_w[f"w{l}b"] = nc.declare_dram_parameter(
            f"w{l}b", [PD[l] // P, P, PD[l + 1]], MM_DT, isOutput=False
        )
    d_b = {}
    for l in range(1, 5):
        d_b[f"b{l}raw"] = nc.declare_dram_parameter(
            f"b{l}raw", [P, PD[l] // P], F32, isOutput=False
        )
        d_b[f"b{l}s"] = nc.declare_dram_parameter(
            f"b{l}s", [P, PD[l] // P], F32, isOutput=False
        )
    d_out = nc.declare_dram_parameter("out", [P, PD[4]], F32, isOutput=True)

    with tile.TileContext(nc) as tc, ExitStack() as ctx:
        st = ctx.enter_context(tc.tile_pool(name="state", bufs=1))
        wp = ctx.enter_context(tc.tile_pool(name="wslab", bufs=4))
        pp = ctx.enter_context(tc.tile_pool(name="psum", bufs=8, space="PSUM"))
        tp = ctx.enter_context(tc.tile_pool(name="tmp", bufs=6))

        # persistent tensors: fp32 master states + fp16 matmul mirrors
        s = {}
        s16 = {}
        for l in range(1, 5):
            s[l] = st.tile([P, PD[l]], F32, tag=f"s{l}", name=f"s{l}")
            s16[l] = st.tile([P, PD[l]], MM_DT, tag=f"s16_{l}", name=f"s16_{l}")
        c1s = st.tile([P, PD[1]], F16, tag="c1s")
        x16 = st.tile([P, PD[0]], MM_DT, tag="x16")
        cx16 = st.tile([P, PD[0]], MM_DT, tag="cx16")
        bias = {}
        for l in range(1, 5):
            bias[f"b{l}raw"] = st.tile(
                [P, PD[l] // P], F32, tag=f"b{l}raw", name=f"b{l}raw"
            )
            bias[f"b{l}s"] = st.tile(
                [P, PD[l] // P], F32, tag=f"b{l}s", name=f"b{l}s"
            )
            nc.sync.dma_start(bias[f"b{l}raw"][:], d_b[f"b{l}raw"][:])
            nc.sync.dma_start(bias[f"b{l}s"][:], d_b[f"b{l}s"][:])

        nc.sync.dma_start(x16[:], d_x16[:])
        nc.sync.dma_start(cx16[:], d_cx16[:])

        def mm_group(psum, slab, rhs16, kt, first, last):
            for k in range(kt):
                nc.tensor.matmul(
                    psum[:],
                    slab[:, bass.ts(k, P)],
                    rhs16[:, bass.ts(k, P)],
                    start=(first and k == 0),
                    stop=(last and k == kt - 1),
                )

        # ---- init pass ----
        # layer 1 init + c1 constant share one pass over w0f
        for m in range(KT[1]):
            wf = wp.tile([P, PD[0]], MM_DT, tag="slab")
            nc.sync.dma_start(wf[:], d_w["w0f"][m])
            ps_i = pp.tile([P, P], F32, tag="ps")
            ps_c = pp.tile([P, P], F32, tag="ps")
            mm_group(ps_i, wf, x16, KT[0], True, True)
            mm_group(ps_c, wf, cx16, KT[0], True, True)
            # s1_init = clip(x @ W0 + b1)
            t = tp.tile([P, P], F32, tag="t")
            nc.vector.tensor_scalar(
                t[:], ps_i[:], bias["b1raw"][:, m : m + 1], 0.0, AL.add, AL.max
            )
            nc.vector.tensor_scalar_min(s[1][:, bass.ts(m, P)], t[:], 1.0)
            nc.gpsimd.tensor_scalar_min(s16[1][:, bass.ts(m, P)], t[:], 1.0)
            # c1s = 0.3 * (clip(x) @ W0 + b1)
            nc.vector.tensor_scalar(
                c1s[:, bass.ts(m, P)],
                ps_c[:],
                0.3,
                bias["b1s"][:, m : m + 1],
                AL.mult,
                AL.add,
            )

        # W3 (smallest matrix) stays resident in SBUF for all sweeps:
        # saves 8 MB/sweep of HBM streaming.
        w3f_res = st.tile([P, KT[4] * PD[3]], MM_DT, tag="w3f_res")
        w3b_res = st.tile([P, KT[3] * PD[4]], MM_DT, tag="w3b_res")
        for m in range(KT[4]):
            nc.sync.dma_start(
                w3f_res[:, m * PD[3] : (m + 1) * PD[3]], d_w["w3f"][m]
            )
        for m in range(KT[3]):
            nc.sync.dma_start(
                w3b_res[:, m * PD[4] : (m + 1) * PD[4]], d_w["w3b"][m]
            )
        # partial residency for W2 backward slabs (as many as SBUF allows)
        N_W2B_RES = 12
        w2b_res = st.tile([P, N_W2B_RES * PD[3]], MM_DT, tag="w2b_res")
        for m in range(N_W2B_RES):
            nc.sync.dma_start(
                w2b_res[:, m * PD[3] : (m + 1) * PD[3]], d_w["w2b"][m]
            )

        # init layers 2..4: s_{l+1} = clip(s_l @ W_l + b_{l+1})
        for l in range(1, 4):
            for m in range(KT[l + 1]):
                if l == 3:
                    wf = w3f_res[:, m * PD[3] : (m + 1) * PD[3]]
                else:
                    wf = wp.tile([P, PD[l]], MM_DT, tag="slab")
                    nc.sync.dma_start(wf[:], d_w[f"w{l}f"][m])
                ps = pp.tile([P, P], F32, tag="ps")
                mm_group(ps, wf, s16[l], KT[l], True, True)
                t = tp.tile([P, P], F32, tag="t")
                nc.vector.tensor_scalar(
                    t[:],
                    ps[:],
                    bias[f"b{l + 1}raw"][:, m : m + 1],
                    0.0,
                    AL.add,
                    AL.max,
                )
                nc.vector.tensor_scalar_min(s[l + 1][:, bass.ts(m, P)], t[:], 1.0)
                nc.gpsimd.tensor_scalar_min(
                    s16[l + 1][:, bass.ts(m, P)], t[:], 1.0
                )

        # ---- relaxation sweeps ----
        # streamed slabs are fetched in adjacent-m pairs (one 1 MB DMA instead
        # of two 0.5 MB ones) for better HBM efficiency
        for _ in range(N_RELAX):
            for l in range(1, 5):
                fwd = None if l == 1 else (d_w[f"w{l - 1}f"], s16[l - 1], KT[l - 1])
                bwd = None if l == 4 else (d_w[f"w{l}b"], s16[l + 1], KT[l + 1])
                pair_f = pair_b = None
                for m in range(KT[l]):
                    if m % 2 == 0:
                        pair_f = pair_b = None
                        if fwd is not None and l != 4:
                            kf = fwd[2] * P
                            pair_f = wp.tile([P, 2 * kf], MM_DT, tag="slab")
                            nc.sync.dma_start(
                                pair_f[:].rearrange("p (i k) -> p i k", i=2),
                                fwd[0][m : m + 2].rearrange("i p k -> p i k"),
                            )
                        if bwd is not None and l != 3 and not (
                            l == 2 and m + 1 < N_W2B_RES
                        ):
                            kb = bwd[2] * P
                            pair_b = wp.tile([P, 2 * kb], MM_DT, tag="slab")
                            nc.sync.dma_start(
                                pair_b[:].rearrange("p (i k) -> p i k", i=2),
                                bwd[0][m : m + 2].rearrange("i p k -> p i k"),
                            )
                    slabs = []
                    if fwd is not None:
                        if l == 4:
                            wf = w3f_res[:, m * PD[3] : (m + 1) * PD[3]]
                        else:
                            kf = fwd[2] * P
                            wf = pair_f[:, (m % 2) * kf : (m % 2 + 1) * kf]
                        slabs.append((wf, fwd[1], fwd[2]))
                    if bwd is not None:
                        if l == 3:
                            wb = w3b_res[:, m * PD[4] : (m + 1) * PD[4]]
                        elif l == 2 and m < N_W2B_RES:
                            wb = w2b_res[:, m * PD[3] : (m + 1) * PD[3]]
                        else:
                            kb = bwd[2] * P
                            wb = pair_b[:, (m % 2) * kb : (m % 2 + 1) * kb]
                        slabs.append((wb, bwd[1], bwd[2]))
                    ps = pp.tile([P, P], F32, tag="ps")
                    for i, (slab, rhs16, kt) in enumerate(slabs):
                        mm_group(ps, slab, rhs16, kt, i == 0, i == len(slabs) - 1)
                    # t = 0.3 * psum + 0.3 * bias   (or + 0.3 * c1 for l=1)
                    t = tp.tile([P, P], F32, tag="t")
                    if l == 1:
                        nc.vector.scalar_tensor_tensor(
                            t[:], ps[:], 0.3, c1s[:, bass.ts(m, P)], AL.mult, AL.add
                        )
                    else:
                        nc.vector.tensor_scalar(
                            t[:], ps[:], 0.3, bias[f"b{l}s"][:, m : m + 1],
                            AL.mult, AL.add,
                        )
                    # u = 0.7 * s + t ; s = clip(u, 0, 1) (fp32 + fp16 mirror)
                    u = tp.tile([P, P], F32, tag="u")
                    nc.vector.scalar_tensor_tensor(
                        u[:], s[l][:, bass.ts(m, P)], 0.7, t[:], AL.mult, AL.add
                    )
                    nc.vector.tensor_scalar(
                        s[l][:, bass.ts(m, P)], u[:], 0.0, 1.0, AL.max, AL.min
                    )
                    nc.gpsimd.tensor_scalar(
                        s16[l][:, bass.ts(m, P)], u[:], 0.0, 1.0, AL.max, AL.min
                    )

        nc.sync.dma_start(d_out[:], s[4][:])

    nc.compile()
    return nc


def _prep_inputs(x, W0, W1, W2, W3, b1, b2, b3, b4):
    """Host-side data prep shared by all cores (weights) + per-core x."""
    common = {
        "w0f": _slab_f(W0, PD[0], PD[1]),
        "w1f": _slab_f(W1, PD[1], PD[2]),
        "w2f": _slab_f(W2, PD[2], PD[3]),
        "w3f": _slab_f(W3, PD[3], PD[4]),
        "w1b": _slab_b(W1, PD[2], PD[1]),
        "w2b": _slab_b(W2, PD[3], PD[2]),
        "w3b": _slab_b(W3, PD[4], PD[3]),
    }
    for l, b in zip(range(1, 5), [b1, b2, b3, b4]):
        common[f"b{l}raw"] = _bias_tiles(b, PD[l], 1.0)
        common[f"b{l}s"] = _bias_tiles(b, PD[l], LR)

    in_maps = []
    for c in range(N_CORES):
        xs = np.asarray(x[c * BPC : (c + 1) * BPC], dtype=np.float32)
        # xT[p, k*P+j] = xs[j, k*P+p]
        xT = np.ascontiguousarray(
            xs.reshape(BPC, PD[0] // P, P).transpose(2, 1, 0)
        ).reshape(P, PD[0])
        in_maps.append({
            "x16T": xT.astype(np.float16),
            "cx16T": np.clip(xT, 0.0, 1.0).astype(np.float16),
            **common,
        })
    return in_maps


_NC_CACHE = None


def _get_nc():
    global _NC_CACHE
    if _NC_CACHE is None:
        _NC_CACHE = build_nc()
    return _NC_CACHE


def decode_output(out_map):
    """Decode {'out': [N_CORES*P, PD[4]]} (or per-core list) to [BATCH, 1000]."""
    o_all = out_map["out"]
    outs = []
    for c in range(N_CORES):
        o = o_all[c * P : (c + 1) * P]  # [P, PD[4]] = [128, 1024]
        # decode: o[p, k*P+j] = s4T[k*P+p, j] = s4[batch j, dim k*P+p]
        s4 = o.reshape(P, PD[4] // P, P).transpose(2, 1, 0).reshape(BPC, PD[4])
        outs.append(s4[:, : DIMS[4]])
    return np.concatenate(outs, axis=0).astype(np.float32)


def run(inputs, trace=False):
    nc = _get_nc()
    in_maps = _prep_inputs(**inputs)
    res = run_bass_kernel_spmd(nc, in_maps, list(range(N_CORES)), trace=trace)
    full = np.concatenate([res.results[c]["out"] for c in range(N_CORES)], axis=0)
    return decode_output({"out": full}), res


def kernel(**inputs):
    out, _ = run(inputs, trace=False)
    return out



# revision 6
# speedup vs baseline: 25.5259x; 1.0228x over previous
"""Trainium2 Bass kernel for nn_EqPropTuned (equilibrium-propagation relaxation).

Network: DIMS = [2048, 2048, 2048, 2048, 1000], BATCH = 1024, 25 Gauss-Seidel
sweeps with lr 0.3, rho = clip(0, 1).

Sharding: data-parallel over batch across 8 cores (128 rows/core), weights
replicated. States live on-chip in dim-major layout [dim, batch_per_core] as
fp16 (fp16-only masters measured within 6e-4 of fp32 masters on this
problem). Matmuls run in fp16 with fp32 PSUM accumulation.

Measured-on-HW design points (see session notes):
  - PE sustains ~49 ns per 128^3 MM at N=128 — same throughput as N=512
    streaming, so the dim-major layout (N = batch = 128) is kept.
  - DMA sustains ~364 GB/s/core even with all 8 cores streaming, so weight
    streaming is sized to stay just under PE time: w2b/w3b/w3f and 4 slabs
    of w2f stay resident in SBUF (18 MB), leaving 22 MB/sweep streamed
    (~60 us) vs ~63 us of PE work per sweep.

Algebraic facts used:
  - states are stored post-clip, so rho() on a stored state is the identity
  - 0.3*(rho(x) @ W0 + b1) is constant across sweeps -> computed once (c1s)
  - forward + backward matmul terms for one state tile accumulate into one
    PSUM group
"""

import os
import numpy as np
from contextlib import ExitStack

import concourse.bass as bass
import concourse.tile as tile
from concourse import mybir, bacc
from concourse.bass_utils import run_bass_kernel_spmd

F32 = mybir.dt.float32
F16 = mybir.dt.float16
AL = mybir.AluOpType
AF = mybir.ActivationFunctionType

P = 128
DIMS = [2048, 2048, 2048, 2048, 1000]
PD = [2048, 2048, 2048, 2048, 1024]  # padded dims
KT = [d // P for d in PD]            # [16, 16, 16, 16, 8] k-tiles per dim
BATCH = 1024
N_CORES = 8
BPC = BATCH // N_CORES               # 128 batch rows per core
N_RELAX = int(os.environ.get("KERNEL_N_RELAX", "25"))
LR = 0.3
N_W2F_RES = 4                        # resident w2f slabs (of 16)

MM_DT = F16
MM_NP = np.float16


def _slab_f(W, Kp, Mp):
    """Forward slabs: out[m, p, k*P+j] = W[k*P+p, m*P+j], shape [Mp/P, P, Kp]."""
    K, M = W.shape
    Wp = np.zeros((Kp, Mp), np.float32)
    Wp[:K, :M] = W
    t = Wp.reshape(Kp // P, P, Mp // P, P)  # [k, p, m, j]
    out = np.ascontiguousarray(t.transpose(2, 1, 0, 3)).reshape(Mp // P, P, Kp)
    return out.astype(MM_NP)


def _slab_b(W, Kp, Mp):
    """Backward slabs built from W.T (contract over W's output dim)."""
    return _slab_f(np.ascontiguousarray(W.T.astype(np.float32)), Kp, Mp)


def _bias_tiles(b, Mp, scale=1.0):
    """[P, Mp/P] with out[p, m] = scale * b[m*P+p]."""
    bp = np.zeros(Mp, np.float32)
    bp[: b.shape[0]] = b * scale
    return np.ascontiguousarray(bp.reshape(Mp // P, P).T)


def build_nc():
    nc = bacc.Bacc(None, target_bir_lowering=False, debug=False)

    d_x16 = nc.declare_dram_parameter("x16T", [P, PD[0]], F16, isOutput=False)
    d_cx16 = nc.declare_dram_parameter("cx16T", [P, PD[0]], F16, isOutput=False)
    d_w = {}
    # streamed slab sets
    d_w["w0f"] = nc.declare_dram_parameter("w0f", [KT[1], P, PD[0]], MM_DT, isOutput=False)
    d_w["w1f"] = nc.declare_dram_parameter("w1f", [KT[2], P, PD[1]], MM_DT, isOutput=False)
    d_w["w1b"] = nc.declare_dram_parameter("w1b", [KT[1], P, PD[2]], MM_DT, isOutput=False)
    d_w["w2f"] = nc.declare_dram_parameter("w2f", [KT[3], P, PD[2]], MM_DT, isOutput=False)
    # resident sets
    d_w["w2b"] = nc.declare_dram_parameter("w2b", [KT[2], P, PD[3]], MM_DT, isOutput=False)
    d_w["w3b"] = nc.declare_dram_parameter("w3b", [KT[3], P, PD[4]], MM_DT, isOutput=False)
    d_w["w3f"] = nc.declare_dram_parameter("w3f", [KT[4], P, PD[3]], MM_DT, isOutput=False)
    d_b = {}
    for l in range(1, 5):
        d_b[f"b{l}raw"] = nc.declare_dram_parameter(
            f"b{l}raw", [P, PD[l] // P], F32, isOutput=False
        )
        d_b[f"b{l}s"] = nc.declare_dram_parameter(
            f"b{l}s", [P, PD[l] // P], F32, isOutput=False
        )
    d_out = nc.declare_dram_parameter("out", [P, PD[4]], F16, isOutput=True)

    with tile.TileContext(nc) as tc, ExitStack() as ctx:
        st = ctx.enter_context(tc.tile_pool(name="state", bufs=1))
        wp = ctx.enter_context(tc.tile_pool(name="wslab", bufs=6))
        pp = ctx.enter_context(tc.tile_pool(name="psum", bufs=8, space="PSUM"))
        tp = ctx.enter_context(tc.tile_pool(name="tmp", bufs=8))

        # persistent fp16 states, dim-major [dim-in-tile, m-tile*P + batch]
        s16 = {}
        for l in range(1, 5):
            s16[l] = st.tile([P, PD[l]], F16, tag=f"s16_{l}", name=f"s16_{l}")
        c1s = st.tile([P, PD[1]], F32, tag="c1s")
        x16 = st.tile([P, PD[0]], F16, tag="x16")
        cx16 = st.tile([P, PD[0]], F16, tag="cx16")
        bias = {}
        for l in range(1, 5):
            bias[f"b{l}raw"] = st.tile(
                [P, PD[l] // P], F32, tag=f"b{l}raw", name=f"b{l}raw"
            )
            bias[f"b{l}s"] = st.tile(
                [P, PD[l] // P], F32, tag=f"b{l}s", name=f"b{l}s"
            )
            nc.sync.dma_start(bias[f"b{l}raw"][:], d_b[f"b{l}raw"][:])
            nc.sync.dma_start(bias[f"b{l}s"][:], d_b[f"b{l}s"][:])

        nc.sync.dma_start(x16[:], d_x16[:])
        nc.sync.dma_start(cx16[:], d_cx16[:])

        def mm_group(psum, slab, rhs16, kt, first, last):
            for k in range(kt):
                nc.tensor.matmul(
                    psum[:],
                    slab[:, bass.ts(k, P)],
                    rhs16[:, bass.ts(k, P)],
                    start=(first and k == 0),
                    stop=(last and k == kt - 1),
                )

        # ---- init pass ----
        # layer 1 init + c1 constant share one pass over w0f
        for m in range(KT[1]):
            wf = wp.tile([P, PD[0]], MM_DT, tag="slab")
            nc.sync.dma_start(wf[:], d_w["w0f"][m])
            ps_i = pp.tile([P, P], F32, tag="ps")
            ps_c = pp.tile([P, P], F32, tag="ps")
            mm_group(ps_i, wf, x16, KT[0], True, True)
            mm_group(ps_c, wf, cx16, KT[0], True, True)
            # s1_init = clip(x @ W0 + b1)
            t = tp.tile([P, P], F32, tag="t")
            nc.scalar.activation(
                t[:], ps_i[:], AF.Identity, bias=bias["b1raw"][:, m : m + 1]
            )
            nc.vector.tensor_scalar(
                s16[1][:, bass.ts(m, P)], t[:], 0.0, 1.0, AL.max, AL.min
            )
            # c1s = 0.3 * (clip(x) @ W0 + b1)
            nc.scalar.activation(
                c1s[:, bass.ts(m, P)], ps_c[:], AF.Identity,
                bias=bias["b1s"][:, m : m + 1], scale=0.3,
            )

        # layer 2 init: s3... uses streamed w1f
        for m in range(KT[2]):
            wf = wp.tile([P, PD[1]], MM_DT, tag="slab")
            nc.sync.dma_start(wf[:], d_w["w1f"][m])
            ps = pp.tile([P, P], F32, tag="ps")
            mm_group(ps, wf, s16[1], KT[1], True, True)
            t = tp.tile([P, P], F32, tag="t")
            nc.scalar.activation(
                t[:], ps[:], AF.Identity, bias=bias["b2raw"][:, m : m + 1]
            )
            nc.vector.tensor_scalar(
                s16[2][:, bass.ts(m, P)], t[:], 0.0, 1.0, AL.max, AL.min
            )

        # resident w2f prefix (used by init l3 and every sweep's l3)
        w2f_res = st.tile([P, N_W2F_RES * PD[2]], MM_DT, tag="w2f_res")
        for m in range(N_W2F_RES):
            nc.sync.dma_start(
                w2f_res[:, m * PD[2] : (m + 1) * PD[2]], d_w["w2f"][m]
            )

        # layer 3 init
        for m in range(KT[3]):
            if m < N_W2F_RES:
                wf = w2f_res[:, m * PD[2] : (m + 1) * PD[2]]
            else:
                wf = wp.tile([P, PD[2]], MM_DT, tag="slab")
                nc.sync.dma_start(wf[:], d_w["w2f"][m])
            ps = pp.tile([P, P], F32, tag="ps")
            mm_group(ps, wf, s16[2], KT[2], True, True)
            t = tp.tile([P, P], F32, tag="t")
            nc.scalar.activation(
                t[:], ps[:], AF.Identity, bias=bias["b3raw"][:, m : m + 1]
            )
            nc.vector.tensor_scalar(
                s16[3][:, bass.ts(m, P)], t[:], 0.0, 1.0, AL.max, AL.min
            )

        # resident w3f (used by init l4 and every sweep's l4)
        w3f_res = st.tile([P, KT[4] * PD[3]], MM_DT, tag="w3f_res")
        for m in range(KT[4]):
            nc.sync.dma_start(
                w3f_res[:, m * PD[3] : (m + 1) * PD[3]], d_w["w3f"][m]
            )

        # layer 4 init
        for m in range(KT[4]):
            wf = w3f_res[:, m * PD[3] : (m + 1) * PD[3]]
            ps = pp.tile([P, P], F32, tag="ps")
            mm_group(ps, wf, s16[3], KT[3], True, True)
            t = tp.tile([P, P], F32, tag="t")
            nc.scalar.activation(
                t[:], ps[:], AF.Identity, bias=bias["b4raw"][:, m : m + 1]
            )
            nc.vector.tensor_scalar(
                s16[4][:, bass.ts(m, P)], t[:], 0.0, 1.0, AL.max, AL.min
            )

        # remaining resident sets (first needed in sweep 1 layers 2/3)
        w2b_res = st.tile([P, KT[2] * PD[3]], MM_DT, tag="w2b_res")
        for m in range(KT[2]):
            nc.sync.dma_start(
                w2b_res[:, m * PD[3] : (m + 1) * PD[3]], d_w["w2b"][m]
            )
        w3b_res = st.tile([P, KT[3] * PD[4]], MM_DT, tag="w3b_res")
        for m in range(KT[3]):
            nc.sync.dma_start(
                w3b_res[:, m * PD[4] : (m + 1) * PD[4]], d_w["w3b"][m]
            )

        # ---- relaxation sweeps ----
        for _ in range(N_RELAX):
            # layer 1: only backward term (fwd term is the c1s constant)
            for m in range(KT[1]):
                wb = wp.tile([P, PD[2]], MM_DT, tag="slab")
                nc.sync.dma_start(wb[:], d_w["w1b"][m])
                ps = pp.tile([P, P], F32, tag="ps")
                mm_group(ps, wb, s16[2], KT[2], True, True)
                t = tp.tile([P, P], F32, tag="t")
                nc.vector.scalar_tensor_tensor(
                    t[:], ps[:], 0.3, c1s[:, bass.ts(m, P)], AL.mult, AL.add
                )
                u = tp.tile([P, P], F32, tag="u")
                nc.vector.scalar_tensor_tensor(
                    u[:], s16[1][:, bass.ts(m, P)], 0.7, t[:], AL.mult, AL.add
                )
                nc.gpsimd.tensor_scalar(
                    s16[1][:, bass.ts(m, P)], u[:], 0.0, 1.0, AL.max, AL.min
                )

            # layer 2: fwd (streamed w1f) + bwd (resident w2b)
            for m in range(KT[2]):
                wf = wp.tile([P, PD[1]], MM_DT, tag="slab")
                nc.sync.dma_start(wf[:], d_w["w1f"][m])
                ps = pp.tile([P, P], F32, tag="ps")
                mm_group(ps, wf, s16[1], KT[1], True, False)
                mm_group(
                    ps, w2b_res[:, m * PD[3] : (m + 1) * PD[3]], s16[3],
                    KT[3], False, True,
                )
                t = tp.tile([P, P], F32, tag="t")
                nc.scalar.activation(
                    t[:], ps[:], AF.Identity, bias=bias["b2s"][:, m : m + 1],
                    scale=0.3,
                )
                u = tp.tile([P, P], F32, tag="u")
                nc.vector.scalar_tensor_tensor(
                    u[:], s16[2][:, bass.ts(m, P)], 0.7, t[:], AL.mult, AL.add
                )
                nc.gpsimd.tensor_scalar(
                    s16[2][:, bass.ts(m, P)], u[:], 0.0, 1.0, AL.max, AL.min
                )

            # layer 3: fwd (resident prefix + streamed w2f) + bwd (resident w3b)
            for m in range(KT[3]):
                if m < N_W2F_RES:
                    wf = w2f_res[:, m * PD[2] : (m + 1) * PD[2]]
                else:
                    wf = wp.tile([P, PD[2]], MM_DT, tag="slab")
                    nc.sync.dma_start(wf[:], d_w["w2f"][m])
                ps = pp.tile([P, P], F32, tag="ps")
                mm_group(ps, wf, s16[2], KT[2], True, False)
                mm_group(
                    ps, w3b_res[:, m * PD[4] : (m + 1) * PD[4]], s16[4],
                    KT[4], False, True,
                )
                t = tp.tile([P, P], F32, tag="t")
                nc.scalar.activation(
                    t[:], ps[:], AF.Identity, bias=bias["b3s"][:, m : m + 1],
                    scale=0.3,
                )
                u = tp.tile([P, P], F32, tag="u")
                nc.vector.scalar_tensor_tensor(
                    u[:], s16[3][:, bass.ts(m, P)], 0.7, t[:], AL.mult, AL.add
                )
                nc.gpsimd.tensor_scalar(
                    s16[3][:, bass.ts(m, P)], u[:], 0.0, 1.0, AL.max, AL.min
                )

            # layer 4: fwd only (resident w3f)
            for m in range(KT[4]):
                wf = w3f_res[:, m * PD[3] : (m + 1) * PD[3]]
                ps = pp.tile([P, P], F32, tag="ps")
                mm_group(ps, wf, s16[3], KT[3], True, True)
                t = tp.tile([P, P], F32, tag="t")
                nc.scalar.activation(
                    t[:], ps[:], AF.Identity, bias=bias["b4s"][:, m : m + 1],
                    scale=0.3,
                )
                u = tp.tile([P, P], F32, tag="u")
                nc.vector.scalar_tensor_tensor(
                    u[:], s16[4][:, bass.ts(m, P)], 0.7, t[:], AL.mult, AL.add
                )
                nc.gpsimd.tensor_scalar(
                    s16[4][:, bass.ts(m, P)], u[:], 0.0, 1.0, AL.max, AL.min
                )

        nc.sync.dma_start(d_out[:], s16[4][:])

    nc.compile()
    return nc


def _prep_inputs(x, W0, W1, W2, W3, b1, b2, b3, b4):
    """Host-side data prep shared by all cores (weights) + per-core x."""
    common = {
        "w0f": _slab_f(W0, PD[0], PD[1]),
        "w1f": _slab_f(W1, PD[1], PD[2]),
        "w2f": _slab_f(W2, PD[2], PD[3]),
        "w3f": _slab_f(W3, PD[3], PD[4]),
        "w1b": _slab_b(W1, PD[2], PD[1]),
        "w2b": _slab_b(W2, PD[3], PD[2]),
        "w3b": _slab_b(W3, PD[4], PD[3]),
    }
    for l, b in zip(range(1, 5), [b1, b2, b3, b4]):
        common[f"b{l}raw"] = _bias_tiles(b, PD[l], 1.0)
        common[f"b{l}s"] = _bias_tiles(b, PD[l], LR)

    in_maps = []
    for c in range(N_CORES):
        xs = np.asarray(x[c * BPC : (c + 1) * BPC], dtype=np.float32)
        # xT[p, k*P+j] = xs[j, k*P+p]
        xT = np.ascontiguousarray(
            xs.reshape(BPC, PD[0] // P, P).transpose(2, 1, 0)
        ).reshape(P, PD[0])
        in_maps.append({
            "x16T": xT.astype(np.float16),
            "cx16T": np.clip(xT, 0.0, 1.0).astype(np.float16),
            **common,
        })
    return in_maps


_NC_CACHE = None


def _get_nc():
    global _NC_CACHE
    if _NC_CACHE is None:
        _NC_CACHE = build_nc()
    return _NC_CACHE


def decode_output(out_map):
    """Decode {'out': [N_CORES*P, PD[4]]} to [BATCH, 1000] float32."""
    o_all = out_map["out"]
    outs = []
    for c in range(N_CORES):
        o = np.asarray(o_all[c * P : (c + 1) * P])  # [P, PD[4]] = [128, 1024]
        # decode: o[p, k*P+j] = s4T[k*P+p, j] = s4[batch j, dim k*P+p]
        s4 = o.reshape(P, PD[4] // P, P).transpose(2, 1, 0).reshape(BPC, PD[4])
        outs.append(s4[:, : DIMS[4]])
    return np.concatenate(outs, axis=0).astype(np.float32)


def run(inputs, trace=False):
    nc = _get_nc()
    in_maps = _prep_inputs(**inputs)
    res = run_bass_kernel_spmd(nc, in_maps, list(range(N_CORES)), trace=trace)
    full = np.concatenate([res.results[c]["out"] for c in range(N_CORES)], axis=0)
    return decode_output({"out": full}), res


def kernel(**inputs):
    out, _ = run(inputs, trace=False)
    return out


# revision 11
# speedup vs baseline: 28.9098x; 1.1326x over previous
"""Trainium2 Bass kernel for nn_EqPropTuned (equilibrium-propagation relaxation).

Network: DIMS = [2048, 2048, 2048, 2048, 1000], BATCH = 1024, 25 Gauss-Seidel
sweeps with lr 0.3, rho = clip(0, 1).

Sharding: data-parallel over batch across 8 cores (128 rows/core), weights
replicated. States live on-chip in dim-major layout [dim, batch_per_core] as
fp16 (fp16-only masters measured within 6e-4 of fp32 masters here). Matmuls
run in fp16 with fp32 PSUM accumulation.

Measured-on-HW design points driving this structure:
  - GPSIMD (Pool) ops cost ~1.3us each on this part -> never used.
  - A 3-op vector update chain paces the whole sweep; so the update
    u = 0.7*s + 0.3*(mm + b) is folded INTO the PSUM accumulation:
      * streamed/resident weight slabs are pre-scaled by 0.3 on the host
      * the 0.7*s term is one extra matmul with stationary 0.7*I
      * biases enter as K=1 rank-1 matmuls (b row) x (ones row);
        layer 1's constant 0.3*(rho(x)@W0+b1) enters via an identity matmul
    leaving a single DVE clip (PSUM -> SBUF fp16) per state tile.
  - DMA sustains ~364 GB/s/core with all 8 cores streaming; weight residency
    is maximized (w2b/w3b/w3f full + 5 w2f slabs = 18.5 MB) so only
    ~21.5 MB/sweep streams, keeping DMA ~60us/sweep.
"""

import os
import numpy as np
from contextlib import ExitStack

import concourse.bass as bass
import concourse.tile as tile
from concourse import mybir, bacc
from concourse.bass_utils import run_bass_kernel_spmd

F32 = mybir.dt.float32
F16 = mybir.dt.float16
AL = mybir.AluOpType
AF = mybir.ActivationFunctionType

P = 128
DIMS = [2048, 2048, 2048, 2048, 1000]
PD = [2048, 2048, 2048, 2048, 1024]  # padded dims
KT = [d // P for d in PD]            # [16, 16, 16, 16, 8] k-tiles per dim
BATCH = 1024
N_CORES = 8
BPC = BATCH // N_CORES               # 128 batch rows per core
N_RELAX = int(os.environ.get("KERNEL_N_RELAX", "25"))
LR = 0.3
N_W2F_RES = 5                        # resident (scaled) w2f slabs of 16

MM_DT = F16
MM_NP = np.float16


def _slab_f(W, Kp, Mp, scale=1.0):
    """Forward slabs: out[m, p, k*P+j] = scale*W[k*P+p, m*P+j] -> [Mp/P, P, Kp]."""
    K, M = W.shape
    Wp = np.zeros((Kp, Mp), np.float32)
    Wp[:K, :M] = W * scale
    t = Wp.reshape(Kp // P, P, Mp // P, P)  # [k, p, m, j]
    out = np.ascontiguousarray(t.transpose(2, 1, 0, 3)).reshape(Mp // P, P, Kp)
    return out.astype(MM_NP)


def _slab_b(W, Kp, Mp, scale=1.0):
    """Backward slabs built from W.T (contract over W's output dim)."""
    return _slab_f(np.ascontiguousarray(W.T.astype(np.float32)), Kp, Mp, scale)


def _bias_row(b, Mp, scale=1.0):
    bp = np.zeros((1, Mp), np.float32)
    bp[0, : b.shape[0]] = b * scale
    return bp.astype(np.float16)


def _bias_tiles(b, Mp, scale=1.0):
    """[P, Mp/P] with out[p, m] = scale * b[m*P+p]."""
    bp = np.zeros(Mp, np.float32)
    bp[: b.shape[0]] = b * scale
    return np.ascontiguousarray(bp.reshape(Mp // P, P).T)


def build_nc():
    nc = bacc.Bacc(None, target_bir_lowering=False, debug=False)

    d_x16 = nc.declare_dram_parameter("x16T", [P, PD[0]], F16, isOutput=False)
    d_cx16 = nc.declare_dram_parameter("cx16T", [P, PD[0]], F16, isOutput=False)
    d_w = {}
    # init-time streamed sets (unscaled)
    d_w["w0u"] = nc.declare_dram_parameter("w0u", [KT[1], P, PD[0]], MM_DT, isOutput=False)
    d_w["w1u"] = nc.declare_dram_parameter("w1u", [KT[2], P, PD[1]], MM_DT, isOutput=False)
    d_w["w2u"] = nc.declare_dram_parameter("w2u", [KT[3], P, PD[2]], MM_DT, isOutput=False)
    # sweep-time streamed sets (0.3-scaled)
    d_w["w1f"] = nc.declare_dram_parameter("w1f", [KT[2], P, PD[1]], MM_DT, isOutput=False)
    d_w["w1b"] = nc.declare_dram_parameter("w1b", [KT[1], P, PD[2]], MM_DT, isOutput=False)
    d_w["w2f"] = nc.declare_dram_parameter("w2f", [KT[3], P, PD[2]], MM_DT, isOutput=False)
    # resident sets (0.3-scaled)
    d_w["w2b"] = nc.declare_dram_parameter("w2b", [KT[2], P, PD[3]], MM_DT, isOutput=False)
    d_w["w3b"] = nc.declare_dram_parameter("w3b", [KT[3], P, PD[4]], MM_DT, isOutput=False)
    d_w["w3f"] = nc.declare_dram_parameter("w3f", [KT[4], P, PD[3]], MM_DT, isOutput=False)
    # identity stationaries
    d_i10 = nc.declare_dram_parameter("i10", [P, P], F16, isOutput=False)
    d_i07 = nc.declare_dram_parameter("i07", [P, P], F16, isOutput=False)
    # sweep bias rows (0.3-scaled, rank-1 matmul operands)
    d_bi = {}
    for l in range(2, 5):
        d_bi[f"b{l}s"] = nc.declare_dram_parameter(f"b{l}s", [1, PD[l]], F16, isOutput=False)
    # init bias column tiles (raw, per-partition) + 0.3-scaled b1 column
    d_bc = {}
    for l in range(1, 5):
        d_bc[f"b{l}c"] = nc.declare_dram_parameter(f"b{l}c", [P, PD[l] // P], F32, isOutput=False)
    d_bc["b1sc"] = nc.declare_dram_parameter("b1sc", [P, PD[1] // P], F32, isOutput=False)
    d_out = nc.declare_dram_parameter("out", [P, PD[4]], F16, isOutput=True)

    with tile.TileContext(nc) as tc, ExitStack() as ctx:
        st = ctx.enter_context(tc.tile_pool(name="state", bufs=1))
        wp = ctx.enter_context(tc.tile_pool(name="wslab", bufs=6))
        pp = ctx.enter_context(tc.tile_pool(name="psum", bufs=8, space="PSUM"))
        tp = ctx.enter_context(tc.tile_pool(name="tmp", bufs=4))

        # persistent fp16 states, dim-major [dim-in-tile, m-tile*P + batch]
        s16 = {}
        for l in range(1, 5):
            s16[l] = st.tile([P, PD[l]], F16, tag=f"s16_{l}", name=f"s16_{l}")
        add1 = st.tile([P, PD[1]], F16, tag="add1")       # 0.3*(rho(x)@W0+b1)
        i10 = st.tile([P, P], F16, tag="i10")
        i07 = st.tile([P, P], F16, tag="i07")
        ones = st.tile([1, P], F16, tag="ones")
        brow = {}
        for l in range(2, 5):
            name = f"b{l}s"
            brow[name] = st.tile([1, PD[l]], F16, tag=name, name=name)
            nc.sync.dma_start(brow[name][:], d_bi[name][:])
        bcol = {}
        for l in range(1, 5):
            bcol[f"b{l}c"] = st.tile([P, PD[l] // P], F32, tag=f"b{l}c", name=f"b{l}c")
            nc.sync.dma_start(bcol[f"b{l}c"][:], d_bc[f"b{l}c"][:])
        bcol["b1sc"] = st.tile([P, PD[1] // P], F32, tag="b1sc", name="b1sc")
        nc.sync.dma_start(bcol["b1sc"][:], d_bc["b1sc"][:])
        nc.sync.dma_start(i10[:], d_i10[:])
        nc.sync.dma_start(i07[:], d_i07[:])
        nc.vector.memset(ones[:], 1.0)
        # x tiles live in the slab pool (init-only)
        x16 = wp.tile([P, PD[0]], F16, tag="slab", name="x16t")
        cx16 = wp.tile([P, PD[0]], F16, tag="slab", name="cx16t")
        nc.sync.dma_start(x16[:], d_x16[:])
        nc.sync.dma_start(cx16[:], d_cx16[:])

        def mm_group(psum, slab, rhs16, kt, first, last):
            for k in range(kt):
                nc.tensor.matmul(
                    psum[:],
                    slab[:, bass.ts(k, P)],
                    rhs16[:, bass.ts(k, P)],
                    start=(first and k == 0),
                    stop=(last and k == kt - 1),
                )

        def bias_mm(psum, row, m, first, last):
            nc.tensor.matmul(
                psum[:], row[:, bass.ts(m, P)], ones[:], start=first, stop=last
            )

        # ---- init pass ----
        # layer 1 init + add1 constant share one pass over w0u
        for m in range(KT[1]):
            wf = wp.tile([P, PD[0]], MM_DT, tag="slab")
            nc.sync.dma_start(wf[:], d_w["w0u"][m])
            ps_i = pp.tile([P, P], F32, tag="ps")
            ps_c = pp.tile([P, P], F32, tag="ps")
            mm_group(ps_i, wf, x16, KT[0], True, True)
            mm_group(ps_c, wf, cx16, KT[0], True, True)
            # s1_init = clip(x @ W0 + b1); add1 = 0.3 * (clip(x) @ W0 + b1)
            t = tp.tile([P, P], F32, tag="t")
            nc.scalar.activation(
                t[:], ps_i[:], AF.Identity, bias=bcol["b1c"][:, m : m + 1]
            )
            nc.vector.tensor_scalar(
                s16[1][:, bass.ts(m, P)], t[:], 0.0, 1.0, AL.max, AL.min
            )
            nc.vector.tensor_scalar(
                add1[:, bass.ts(m, P)], ps_c[:], 0.3, bcol["b1sc"][:, m : m + 1],
                AL.mult, AL.add,
            )

        # layer 2 init (streamed unscaled w1)
        for m in range(KT[2]):
            wf = wp.tile([P, PD[1]], MM_DT, tag="slab")
            nc.sync.dma_start(wf[:], d_w["w1u"][m])
            ps = pp.tile([P, P], F32, tag="ps")
            mm_group(ps, wf, s16[1], KT[1], True, True)
            t = tp.tile([P, P], F32, tag="t")
            nc.scalar.activation(
                t[:], ps[:], AF.Identity, bias=bcol["b2c"][:, m : m + 1]
            )
            nc.vector.tensor_scalar(
                s16[2][:, bass.ts(m, P)], t[:], 0.0, 1.0, AL.max, AL.min
            )

        # layer 3 init (streamed unscaled w2)
        for m in range(KT[3]):
            wf = wp.tile([P, PD[2]], MM_DT, tag="slab")
            nc.sync.dma_start(wf[:], d_w["w2u"][m])
            ps = pp.tile([P, P], F32, tag="ps")
            mm_group(ps, wf, s16[2], KT[2], True, True)
            t = tp.tile([P, P], F32, tag="t")
            nc.scalar.activation(
                t[:], ps[:], AF.Identity, bias=bcol["b3c"][:, m : m + 1]
            )
            nc.vector.tensor_scalar(
                s16[3][:, bass.ts(m, P)], t[:], 0.0, 1.0, AL.max, AL.min
            )

        # resident (scaled) weight loads
        w3f_res = st.tile([P, KT[4] * PD[3]], MM_DT, tag="w3f_res")
        for m in range(KT[4]):
            nc.sync.dma_start(
                w3f_res[:, m * PD[3] : (m + 1) * PD[3]], d_w["w3f"][m]
            )

        # layer 4 init using scaled resident w3f: s4 = clip(ps/0.3 + b4)
        for m in range(KT[4]):
            wf = w3f_res[:, m * PD[3] : (m + 1) * PD[3]]
            ps = pp.tile([P, P], F32, tag="ps")
            mm_group(ps, wf, s16[3], KT[3], True, True)
            t = tp.tile([P, P], F32, tag="t")
            nc.vector.tensor_scalar(
                t[:], ps[:], 1.0 / 0.3, bcol["b4c"][:, m : m + 1], AL.mult, AL.add
            )
            nc.vector.tensor_scalar(
                s16[4][:, bass.ts(m, P)], t[:], 0.0, 1.0, AL.max, AL.min
            )

        w2b_res = st.tile([P, KT[2] * PD[3]], MM_DT, tag="w2b_res")
        for m in range(KT[2]):
            nc.sync.dma_start(
                w2b_res[:, m * PD[3] : (m + 1) * PD[3]], d_w["w2b"][m]
            )
        w3b_res = st.tile([P, KT[3] * PD[4]], MM_DT, tag="w3b_res")
        for m in range(KT[3]):
            nc.sync.dma_start(
                w3b_res[:, m * PD[4] : (m + 1) * PD[4]], d_w["w3b"][m]
            )
        w2f_res = st.tile([P, N_W2F_RES * PD[2]], MM_DT, tag="w2f_res")
        for m in range(N_W2F_RES):
            nc.sync.dma_start(
                w2f_res[:, m * PD[2] : (m + 1) * PD[2]], d_w["w2f"][m]
            )

        # ---- relaxation sweeps ----
        # per tile: psum accumulates 0.3*mm-terms (pre-scaled slabs)
        #   + 0.7*s (i07 matmul) + constant (add1 identity-mm / bias rank-1 mm)
        # then one DVE clip writes the new fp16 state.
        for _ in range(N_RELAX):
            # layer 1: constant add1 + 0.7*s1 + bwd (streamed scaled w1b)
            for m in range(KT[1]):
                wb = wp.tile([P, PD[2]], MM_DT, tag="slab")
                nc.sync.dma_start(wb[:], d_w["w1b"][m])
                ps = pp.tile([P, P], F32, tag="ps")
                nc.tensor.matmul(
                    ps[:], i10[:], add1[:, bass.ts(m, P)], start=True, stop=False
                )
                nc.tensor.matmul(
                    ps[:], i07[:], s16[1][:, bass.ts(m, P)], start=False, stop=False
                )
                mm_group(ps, wb, s16[2], KT[2], False, True)
                nc.vector.tensor_scalar(
                    s16[1][:, bass.ts(m, P)], ps[:], 0.0, 1.0, AL.max, AL.min
                )

            # layer 2: bias + 0.7*s2 + fwd (streamed w1f) + bwd (resident w2b)
            for m in range(KT[2]):
                wf = wp.tile([P, PD[1]], MM_DT, tag="slab")
                nc.sync.dma_start(wf[:], d_w["w1f"][m])
                ps = pp.tile([P, P], F32, tag="ps")
                bias_mm(ps, brow["b2s"], m, True, False)
                nc.tensor.matmul(
                    ps[:], i07[:], s16[2][:, bass.ts(m, P)], start=False, stop=False
                )
                mm_group(ps, wf, s16[1], KT[1], False, False)
                mm_group(
                    ps, w2b_res[:, m * PD[3] : (m + 1) * PD[3]], s16[3],
                    KT[3], False, True,
                )
                nc.vector.tensor_scalar(
                    s16[2][:, bass.ts(m, P)], ps[:], 0.0, 1.0, AL.max, AL.min
                )

            # layer 3: bias + 0.7*s3 + fwd (res prefix + streamed w2f) + bwd (res w3b)
            for m in range(KT[3]):
                if m < N_W2F_RES:
                    wf = w2f_res[:, m * PD[2] : (m + 1) * PD[2]]
                else:
                    wf = wp.tile([P, PD[2]], MM_DT, tag="slab")
                    nc.sync.dma_start(wf[:], d_w["w2f"][m])
                ps = pp.tile([P, P], F32, tag="ps")
                bias_mm(ps, brow["b3s"], m, True, False)
                nc.tensor.matmul(
                    ps[:], i07[:], s16[3][:, bass.ts(m, P)], start=False, stop=False
                )
                mm_group(ps, wf, s16[2], KT[2], False, False)
                mm_group(
                    ps, w3b_res[:, m * PD[4] : (m + 1) * PD[4]], s16[4],
                    KT[4], False, True,
                )
                nc.vector.tensor_scalar(
                    s16[3][:, bass.ts(m, P)], ps[:], 0.0, 1.0, AL.max, AL.min
                )

            # layer 4: bias + 0.7*s4 + fwd (resident w3f)
            for m in range(KT[4]):
                wf = w3f_res[:, m * PD[3] : (m + 1) * PD[3]]
                ps = pp.tile([P, P], F32, tag="ps")
                bias_mm(ps, brow["b4s"], m, True, False)
                nc.tensor.matmul(
                    ps[:], i07[:], s16[4][:, bass.ts(m, P)], start=False, stop=False
                )
                mm_group(ps, wf, s16[3], KT[3], False, True)
                nc.vector.tensor_scalar(
                    s16[4][:, bass.ts(m, P)], ps[:], 0.0, 1.0, AL.max, AL.min
                )

        nc.sync.dma_start(d_out[:], s16[4][:])

    nc.compile()
    return nc


def _prep_inputs(x, W0, W1, W2, W3, b1, b2, b3, b4):
    """Host-side data prep shared by all cores (weights) + per-core x."""
    common = {
        "w0u": _slab_f(W0, PD[0], PD[1]),
        "w1u": _slab_f(W1, PD[1], PD[2]),
        "w2u": _slab_f(W2, PD[2], PD[3]),
        "w1f": _slab_f(W1, PD[1], PD[2], LR),
        "w2f": _slab_f(W2, PD[2], PD[3], LR),
        "w3f": _slab_f(W3, PD[3], PD[4], LR),
        "w1b": _slab_b(W1, PD[2], PD[1], LR),
        "w2b": _slab_b(W2, PD[3], PD[2], LR),
        "w3b": _slab_b(W3, PD[4], PD[3], LR),
        "i10": np.eye(P, dtype=np.float16),
        "i07": (0.7 * np.eye(P)).astype(np.float16),
        "b1sc": _bias_tiles(np.asarray(b1, np.float32), PD[1], LR),
    }
    for l, b in zip(range(1, 5), [b1, b2, b3, b4]):
        common[f"b{l}c"] = _bias_tiles(np.asarray(b, np.float32), PD[l], 1.0)
        if l >= 2:
            common[f"b{l}s"] = _bias_row(np.asarray(b, np.float32), PD[l], LR)

    in_maps = []
    for c in range(N_CORES):
        xs = np.asarray(x[c * BPC : (c + 1) * BPC], dtype=np.float32)
        # xT[p, k*P+j] = xs[j, k*P+p]
        xT = np.ascontiguousarray(
            xs.reshape(BPC, PD[0] // P, P).transpose(2, 1, 0)
        ).reshape(P, PD[0])
        in_maps.append({
            "x16T": xT.astype(np.float16),
            "cx16T": np.clip(xT, 0.0, 1.0).astype(np.float16),
            **common,
        })
    return in_maps


_NC_CACHE = None


def _get_nc():
    global _NC_CACHE
    if _NC_CACHE is None:
        _NC_CACHE = build_nc()
    return _NC_CACHE


def decode_output(out_map):
    """Decode {'out': [N_CORES*P, PD[4]]} to [BATCH, 1000] float32."""
    o_all = out_map["out"]
    outs = []
    for c in range(N_CORES):
        o = np.asarray(o_all[c * P : (c + 1) * P])  # [P, PD[4]] = [128, 1024]
        # decode: o[p, k*P+j] = s4T[k*P+p, j] = s4[batch j, dim k*P+p]
        s4 = o.reshape(P, PD[4] // P, P).transpose(2, 1, 0).reshape(BPC, PD[4])
        outs.append(s4[:, : DIMS[4]])
    return np.concatenate(outs, axis=0).astype(np.float32)


def run(inputs, trace=False):
    nc = _get_nc()
    in_maps = _prep_inputs(**inputs)
    res = run_bass_kernel_spmd(nc, in_maps, list(range(N_CORES)), trace=trace)
    full = np.concatenate([res.results[c]["out"] for c in range(N_CORES)], axis=0)
    return decode_output({"out": full}), res


def kernel(**inputs):
    out, _ = run(inputs, trace=False)
    return out


# revision 13
# speedup vs baseline: 29.5786x; 1.0231x over previous
"""Trainium2 Bass kernel for nn_EqPropTuned (equilibrium-propagation relaxation).

Network: DIMS = [2048, 2048, 2048, 2048, 1000], BATCH = 1024, 25 Gauss-Seidel
sweeps with lr 0.3, rho = clip(0, 1).

Sharding: data-parallel over batch across 8 cores (128 rows/core), weights
replicated. States live on-chip in dim-major layout [dim, batch_per_core] as
fp16 (fp16-only masters measured within 6e-4 of fp32 masters here). Matmuls
run in fp16 with fp32 PSUM accumulation.

Measured-on-HW design points driving this structure:
  - GPSIMD (Pool) ops cost ~1.3us each on this part -> never used.
  - A 3-op vector update chain paces the whole sweep; so the update
    u = 0.7*s + 0.3*(mm + b) is folded INTO the PSUM accumulation:
      * streamed/resident weight slabs are pre-scaled by 0.3 on the host
      * the 0.7*s term is one extra matmul with stationary 0.7*I
      * biases enter as K=1 rank-1 matmuls (b row) x (ones row);
        layer 1's constant 0.3*(rho(x)@W0+b1) enters via an identity matmul
    leaving a single DVE clip (PSUM -> SBUF fp16) per state tile.
  - DMA sustains ~364 GB/s/core with all 8 cores streaming; weight residency
    is maximized (w2b/w3b/w3f full + w2f prefix) so only ~22 MB/sweep
    streams, keeping DMA ~60us/sweep.
  - Per-call runtime overhead scales with the NUMBER of kernel parameters
    (~35us each), so all inputs are packed into 3 DRAM tensors.
"""

import os
import numpy as np
from contextlib import ExitStack

import concourse.bass as bass
import concourse.tile as tile
from concourse import mybir, bacc
from concourse.bass_utils import run_bass_kernel_spmd

F32 = mybir.dt.float32
F16 = mybir.dt.float16
AL = mybir.AluOpType
AF = mybir.ActivationFunctionType

P = 128
DIMS = [2048, 2048, 2048, 2048, 1000]
PD = [2048, 2048, 2048, 2048, 1024]  # padded dims
KT = [d // P for d in PD]            # [16, 16, 16, 16, 8] k-tiles per dim
BATCH = 1024
N_CORES = 8
BPC = BATCH // N_CORES               # 128 batch rows per core
N_RELAX = int(os.environ.get("KERNEL_N_RELAX", "25"))
LR = 0.3
N_W2F_RES = 4                        # resident (scaled) w2f slabs of 16

MM_DT = F16
MM_NP = np.float16

# wpack row offsets: 7 sets of 16 rows + w3f (8) + w3b packed (8)
WOFF = {"w0u": 0, "w1u": 16, "w2u": 32, "w1f": 48, "w1b": 64, "w2f": 80,
        "w2b": 96, "w3f": 112, "w3bp": 120}
WROWS = 128


def _slab_f(W, Kp, Mp, scale=1.0):
    """Forward slabs: out[m, p, k*P+j] = scale*W[k*P+p, m*P+j] -> [Mp/P, P, Kp]."""
    K, M = W.shape
    Wp = np.zeros((Kp, Mp), np.float32)
    Wp[:K, :M] = W * scale
    t = Wp.reshape(Kp // P, P, Mp // P, P)  # [k, p, m, j]
    out = np.ascontiguousarray(t.transpose(2, 1, 0, 3)).reshape(Mp // P, P, Kp)
    return out.astype(MM_NP)


def _slab_b(W, Kp, Mp, scale=1.0):
    """Backward slabs built from W.T (contract over W's output dim)."""
    return _slab_f(np.ascontiguousarray(W.T.astype(np.float32)), Kp, Mp, scale)


def _bias_tiles(b, Mp, scale=1.0):
    """[P, Mp/P] with out[p, m] = scale * b[m*P+p]."""
    bp = np.zeros(Mp, np.float32)
    bp[: b.shape[0]] = b * scale
    return np.ascontiguousarray(bp.reshape(Mp // P, P).T)


def build_nc():
    nc = bacc.Bacc(None, target_bir_lowering=False, debug=False)

    d_w = nc.declare_dram_parameter("wpack", [WROWS, P, 2048], F16, isOutput=False)
    # mpack rows: 0=x16T, 1=cx16T, 2=[i10|i07|...], 3=bias rows
    #   row 3 partition 0 cols 0:2048   = b2s (0.3*b2)
    #   row 3 partition 1 cols 0:2048   = b3s
    #   row 3 partition 2 cols 0:1024   = b4s
    d_m = nc.declare_dram_parameter("mpack", [4, P, 2048], F16, isOutput=False)
    # cpack cols: [0:16]=b1c, [16:32]=b2c, [32:48]=b3c, [48:64]=b4c, [64:80]=b1sc
    d_c = nc.declare_dram_parameter("cpack", [P, 80], F32, isOutput=False)
    d_out = nc.declare_dram_parameter("out", [P, PD[4]], F16, isOutput=True)

    def wslice(name, m):
        if name == "w3b":
            return d_w[WOFF["w3bp"] + m // 2, :, (m % 2) * PD[4]:(m % 2 + 1) * PD[4]]
        return d_w[WOFF[name] + m]

    with tile.TileContext(nc) as tc, ExitStack() as ctx:
        st = ctx.enter_context(tc.tile_pool(name="state", bufs=1))
        wp = ctx.enter_context(tc.tile_pool(name="wslab", bufs=6))
        pp = ctx.enter_context(tc.tile_pool(name="psum", bufs=8, space="PSUM"))
        tp = ctx.enter_context(tc.tile_pool(name="tmp", bufs=4))

        # persistent fp16 states, dim-major [dim-in-tile, m-tile*P + batch]
        s16 = {}
        for l in range(1, 5):
            s16[l] = st.tile([P, PD[l]], F16, tag=f"s16_{l}", name=f"s16_{l}")
        add1 = st.tile([P, PD[1]], F16, tag="add1")       # 0.3*(rho(x)@W0+b1)
        ii = st.tile([P, 2 * P], F16, tag="ii")           # [i10 | i07]
        ones = st.tile([1, P], F16, tag="ones")
        brow = {
            "b2s": st.tile([1, PD[2]], F16, tag="b2s", name="b2s"),
            "b3s": st.tile([1, PD[3]], F16, tag="b3s", name="b3s"),
            "b4s": st.tile([1, PD[4]], F16, tag="b4s", name="b4s"),
        }
        nc.sync.dma_start(brow["b2s"][:], d_m[3, 0:1, :])
        nc.sync.dma_start(brow["b3s"][:], d_m[3, 1:2, :])
        nc.sync.dma_start(brow["b4s"][:], d_m[3, 2:3, :PD[4]])
        bcolt = st.tile([P, 80], F32, tag="bcolt")
        nc.sync.dma_start(bcolt[:], d_c[:])
        bcol = {"b1c": bcolt[:, 0:16], "b2c": bcolt[:, 16:32],
                "b3c": bcolt[:, 32:48], "b4c": bcolt[:, 48:64],
                "b1sc": bcolt[:, 64:80]}
        nc.sync.dma_start(ii[:], d_m[2, :, : 2 * P])
        i10 = ii[:, 0:P]
        i07 = ii[:, P:2 * P]
        nc.vector.memset(ones[:], 1.0)
        # x tiles live in the slab pool (init-only)
        x16 = wp.tile([P, PD[0]], F16, tag="slab", name="x16t")
        cx16 = wp.tile([P, PD[0]], F16, tag="slab", name="cx16t")
        nc.sync.dma_start(x16[:], d_m[0])
        nc.sync.dma_start(cx16[:], d_m[1])

        def mm_group(psum, slab, rhs16, kt, first, last):
            for k in range(kt):
                nc.tensor.matmul(
                    psum[:],
                    slab[:, bass.ts(k, P)],
                    rhs16[:, bass.ts(k, P)],
                    start=(first and k == 0),
                    stop=(last and k == kt - 1),
                )

        def bias_mm(psum, row, m, first, last):
            nc.tensor.matmul(
                psum[:], row[:, bass.ts(m, P)], ones[:], start=first, stop=last
            )

        # ---- init pass ----
        # layer 1 init + add1 constant share one pass over w0u
        for m in range(KT[1]):
            wf = wp.tile([P, PD[0]], MM_DT, tag="slab")
            nc.sync.dma_start(wf[:], wslice("w0u", m))
            ps_i = pp.tile([P, P], F32, tag="ps")
            ps_c = pp.tile([P, P], F32, tag="ps")
            mm_group(ps_i, wf, x16, KT[0], True, True)
            mm_group(ps_c, wf, cx16, KT[0], True, True)
            # s1_init = clip(x @ W0 + b1); add1 = 0.3 * (clip(x) @ W0 + b1)
            t = tp.tile([P, P], F32, tag="t")
            nc.scalar.activation(
                t[:], ps_i[:], AF.Identity, bias=bcol["b1c"][:, m : m + 1]
            )
            nc.vector.tensor_scalar(
                s16[1][:, bass.ts(m, P)], t[:], 0.0, 1.0, AL.max, AL.min
            )
            nc.vector.tensor_scalar(
                add1[:, bass.ts(m, P)], ps_c[:], 0.3, bcol["b1sc"][:, m : m + 1],
                AL.mult, AL.add,
            )

        # layer 2 init (streamed unscaled w1)
        for m in range(KT[2]):
            wf = wp.tile([P, PD[1]], MM_DT, tag="slab")
            nc.sync.dma_start(wf[:], wslice("w1u", m))
            ps = pp.tile([P, P], F32, tag="ps")
            mm_group(ps, wf, s16[1], KT[1], True, True)
            t = tp.tile([P, P], F32, tag="t")
            nc.scalar.activation(
                t[:], ps[:], AF.Identity, bias=bcol["b2c"][:, m : m + 1]
            )
            nc.vector.tensor_scalar(
                s16[2][:, bass.ts(m, P)], t[:], 0.0, 1.0, AL.max, AL.min
            )

        # layer 3 init (streamed unscaled w2)
        for m in range(KT[3]):
            wf = wp.tile([P, PD[2]], MM_DT, tag="slab")
            nc.sync.dma_start(wf[:], wslice("w2u", m))
            ps = pp.tile([P, P], F32, tag="ps")
            mm_group(ps, wf, s16[2], KT[2], True, True)
            t = tp.tile([P, P], F32, tag="t")
            nc.scalar.activation(
                t[:], ps[:], AF.Identity, bias=bcol["b3c"][:, m : m + 1]
            )
            nc.vector.tensor_scalar(
                s16[3][:, bass.ts(m, P)], t[:], 0.0, 1.0, AL.max, AL.min
            )

        # resident (scaled) weight loads
        w3f_res = st.tile([P, KT[4] * PD[3]], MM_DT, tag="w3f_res")
        for m in range(KT[4]):
            nc.sync.dma_start(
                w3f_res[:, m * PD[3] : (m + 1) * PD[3]], wslice("w3f", m)
            )

        # layer 4 init using scaled resident w3f: s4 = clip(ps/0.3 + b4)
        for m in range(KT[4]):
            wf = w3f_res[:, m * PD[3] : (m + 1) * PD[3]]
            ps = pp.tile([P, P], F32, tag="ps")
            mm_group(ps, wf, s16[3], KT[3], True, True)
            t = tp.tile([P, P], F32, tag="t")
            nc.vector.tensor_scalar(
                t[:], ps[:], 1.0 / 0.3, bcol["b4c"][:, m : m + 1], AL.mult, AL.add
            )
            nc.vector.tensor_scalar(
                s16[4][:, bass.ts(m, P)], t[:], 0.0, 1.0, AL.max, AL.min
            )

        w2b_res = st.tile([P, KT[2] * PD[3]], MM_DT, tag="w2b_res")
        for m in range(KT[2]):
            nc.sync.dma_start(
                w2b_res[:, m * PD[3] : (m + 1) * PD[3]], wslice("w2b", m)
            )
        w3b_res = st.tile([P, KT[3] * PD[4]], MM_DT, tag="w3b_res")
        for m in range(KT[3]):
            nc.sync.dma_start(
                w3b_res[:, m * PD[4] : (m + 1) * PD[4]], wslice("w3b", m)
            )
        w2f_res = st.tile([P, N_W2F_RES * PD[2]], MM_DT, tag="w2f_res")
        for m in range(N_W2F_RES):
            nc.sync.dma_start(
                w2f_res[:, m * PD[2] : (m + 1) * PD[2]], wslice("w2f", m)
            )

        # ---- relaxation sweeps ----
        # per tile: psum accumulates 0.3*mm-terms (pre-scaled slabs)
        #   + 0.7*s (i07 matmul) + constant (add1 identity-mm / bias rank-1 mm)
        # then one DVE clip writes the new fp16 state.
        for _ in range(N_RELAX):
            # layer 1: constant add1 + 0.7*s1 + bwd (streamed scaled w1b)
            for m in range(KT[1]):
                wb = wp.tile([P, PD[2]], MM_DT, tag="slab")
                nc.sync.dma_start(wb[:], wslice("w1b", m))
                ps = pp.tile([P, P], F32, tag="ps")
                nc.tensor.matmul(
                    ps[:], i10, add1[:, bass.ts(m, P)], start=True, stop=False
                )
                nc.tensor.matmul(
                    ps[:], i07, s16[1][:, bass.ts(m, P)], start=False, stop=False
                )
                mm_group(ps, wb, s16[2], KT[2], False, True)
                nc.vector.tensor_scalar(
                    s16[1][:, bass.ts(m, P)], ps[:], 0.0, 1.0, AL.max, AL.min
                )

            # layer 2: bias + 0.7*s2 + fwd (streamed w1f) + bwd (resident w2b)
            for m in range(KT[2]):
                wf = wp.tile([P, PD[1]], MM_DT, tag="slab")
                nc.sync.dma_start(wf[:], wslice("w1f", m))
                ps = pp.tile([P, P], F32, tag="ps")
                bias_mm(ps, brow["b2s"], m, True, False)
                nc.tensor.matmul(
                    ps[:], i07, s16[2][:, bass.ts(m, P)], start=False, stop=False
                )
                mm_group(ps, wf, s16[1], KT[1], False, False)
                mm_group(
                    ps, w2b_res[:, m * PD[3] : (m + 1) * PD[3]], s16[3],
                    KT[3], False, True,
                )
                nc.vector.tensor_scalar(
                    s16[2][:, bass.ts(m, P)], ps[:], 0.0, 1.0, AL.max, AL.min
                )

            # layer 3: bias + 0.7*s3 + fwd (res prefix + streamed w2f) + bwd (res w3b)
            for m in range(KT[3]):
                if m < N_W2F_RES:
                    wf = w2f_res[:, m * PD[2] : (m + 1) * PD[2]]
                else:
                    wf = wp.tile([P, PD[2]], MM_DT, tag="slab")
                    nc.sync.dma_start(wf[:], wslice("w2f", m))
                ps = pp.tile([P, P], F32, tag="ps")
                bias_mm(ps, brow["b3s"], m, True, False)
                nc.tensor.matmul(
                    ps[:], i07, s16[3][:, bass.ts(m, P)], start=False, stop=False
                )
                mm_group(ps, wf, s16[2], KT[2], False, False)
                mm_group(
                    ps, w3b_res[:, m * PD[4] : (m + 1) * PD[4]], s16[4],
                    KT[4], False, True,
                )
                nc.vector.tensor_scalar(
                    s16[3][:, bass.ts(m, P)], ps[:], 0.0, 1.0, AL.max, AL.min
                )

            # layer 4: bias + 0.7*s4 + fwd (resident w3f)
            for m in range(KT[4]):
                wf = w3f_res[:, m * PD[3] : (m + 1) * PD[3]]
                ps = pp.tile([P, P], F32, tag="ps")
                bias_mm(ps, brow["b4s"], m, True, False)
                nc.tensor.matmul(
                    ps[:], i07, s16[4][:, bass.ts(m, P)], start=False, stop=False
                )
                mm_group(ps, wf, s16[3], KT[3], False, True)
                nc.vector.tensor_scalar(
                    s16[4][:, bass.ts(m, P)], ps[:], 0.0, 1.0, AL.max, AL.min
                )

        nc.sync.dma_start(d_out[:], s16[4][:])

    nc.compile()
    return nc


def _prep_inputs(x, W0, W1, W2, W3, b1, b2, b3, b4):
    """Host-side data prep shared by all cores (weights) + per-core x."""
    wpack = np.zeros((WROWS, P, 2048), MM_NP)
    wpack[WOFF["w0u"]:WOFF["w0u"] + 16] = _slab_f(W0, PD[0], PD[1])
    wpack[WOFF["w1u"]:WOFF["w1u"] + 16] = _slab_f(W1, PD[1], PD[2])
    wpack[WOFF["w2u"]:WOFF["w2u"] + 16] = _slab_f(W2, PD[2], PD[3])
    wpack[WOFF["w1f"]:WOFF["w1f"] + 16] = _slab_f(W1, PD[1], PD[2], LR)
    wpack[WOFF["w1b"]:WOFF["w1b"] + 16] = _slab_b(W1, PD[2], PD[1], LR)
    wpack[WOFF["w2f"]:WOFF["w2f"] + 16] = _slab_f(W2, PD[2], PD[3], LR)
    wpack[WOFF["w2b"]:WOFF["w2b"] + 16] = _slab_b(W2, PD[3], PD[2], LR)
    wpack[WOFF["w3f"]:WOFF["w3f"] + 8] = _slab_f(W3, PD[3], PD[4], LR)
    w3b = _slab_b(W3, PD[4], PD[3], LR)  # [16, P, 1024]
    wpack[WOFF["w3bp"]:WOFF["w3bp"] + 8] = w3b.reshape(8, 2, P, PD[4]).transpose(
        0, 2, 1, 3).reshape(8, P, 2048)

    cpack = np.zeros((P, 80), np.float32)
    for i, (b, pd) in enumerate(zip([b1, b2, b3, b4], PD[1:])):
        cpack[:, i * 16 : i * 16 + pd // P] = _bias_tiles(
            np.asarray(b, np.float32), pd, 1.0
        )
    cpack[:, 64:80] = _bias_tiles(np.asarray(b1, np.float32), PD[1], LR)

    mrow3 = np.zeros((P, 2048), MM_NP)
    mrow3[0, :] = (np.pad(np.asarray(b2, np.float32), (0, PD[2] - len(b2)))
                   * LR).astype(MM_NP)
    mrow3[1, :] = (np.pad(np.asarray(b3, np.float32), (0, PD[3] - len(b3)))
                   * LR).astype(MM_NP)
    mrow3[2, :PD[4]] = (np.pad(np.asarray(b4, np.float32), (0, PD[4] - len(b4)))
                        * LR).astype(MM_NP)
    mrow2 = np.zeros((P, 2048), MM_NP)
    mrow2[:, :P] = np.eye(P, dtype=MM_NP)
    mrow2[:, P:2 * P] = (0.7 * np.eye(P)).astype(MM_NP)

    in_maps = []
    for c in range(N_CORES):
        xs = np.asarray(x[c * BPC : (c + 1) * BPC], dtype=np.float32)
        # xT[p, k*P+j] = xs[j, k*P+p]
        xT = np.ascontiguousarray(
            xs.reshape(BPC, PD[0] // P, P).transpose(2, 1, 0)
        ).reshape(P, PD[0])
        mpack = np.stack([
            xT.astype(MM_NP),
            np.clip(xT, 0.0, 1.0).astype(MM_NP),
            mrow2,
            mrow3,
        ])
        in_maps.append({"wpack": wpack, "mpack": mpack, "cpack": cpack})
    return in_maps


_NC_CACHE = None


def _get_nc():
    global _NC_CACHE
    if _NC_CACHE is None:
        _NC_CACHE = build_nc()
    return _NC_CACHE


def decode_output(out_map):
    """Decode {'out': [N_CORES*P, PD[4]]} to [BATCH, 1000] float32."""
    o_all = out_map["out"]
    outs = []
    for c in range(N_CORES):
        o = np.asarray(o_all[c * P : (c + 1) * P])  # [P, PD[4]] = [128, 1024]
        # decode: o[p, k*P+j] = s4T[k*P+p, j] = s4[batch j, dim k*P+p]
        s4 = o.reshape(P, PD[4] // P, P).transpose(2, 1, 0).reshape(BPC, PD[4])
        outs.append(s4[:, : DIMS[4]])
    return np.concatenate(outs, axis=0).astype(np.float32)


def run(inputs, trace=False):
    nc = _get_nc()
    in_maps = _prep_inputs(**inputs)
    res = run_bass_kernel_spmd(nc, in_maps, list(range(N_CORES)), trace=trace)
    full = np.concatenate([res.results[c]["out"] for c in range(N_CORES)], axis=0)
    return decode_output({"out": full}), res


def kernel(**inputs):
    out, _ = run(inputs, trace=False)
    return out


# revision 15
# speedup vs baseline: 30.3010x; 1.0244x over previous
"""Trainium2 Bass kernel for nn_EqPropTuned (equilibrium-propagation relaxation).

Network: DIMS = [2048, 2048, 2048, 2048, 1000], BATCH = 1024, 25 Gauss-Seidel
sweeps with lr 0.3, rho = clip(0, 1).

Sharding: data-parallel over batch across 8 cores (128 rows/core), weights
replicated, fp16 states/weights with fp32 PSUM accumulation (fp16-only state
masters measured within 6e-4 of fp32 masters on this problem).

Structure (all choices A/B-measured on this part):
  - FLIPPED matmul layout: stationary = dim-major state tile [128 dim, 128
    batch] reused across four N=512 matmuls whose moving operand is a weight
    panel [128 dim, 512 out]. Measured 1.7x faster per sweep than the
    classic dim-major N=128 layout (the per-matmul LDWEIGHTS cost
    dominates there; walrus runs with ldw-opt disabled).
  - The whole update u = 0.7*s + 0.3*(mm + b) is folded into the PSUM
    accumulation: weights pre-scaled by 0.3 on the host, the 0.7*s term via
    identity matmuls into 128-col psum slices, biases via K=1 rank-1
    matmuls (ones x bias-row). One DVE clip per 512-wide block writes the
    batch-major fp16 state, then four PE transposes + ACT copies restore
    the dim-major state tiles. (GPSIMD is never used: ~1.3us/op here.)
  - Updated states feed the next layer per Gauss-Seidel; transposes are
    software-pipelined one block behind the matmul stream.
  - Weight residency maximized (w2b/w3b/w3f full + first w2f panel) so only
    ~22 MB/sweep streams (DMA measured ~364 GB/s/core, fully hidden).
  - All inputs packed into 2 DRAM tensors (per-parameter per-call runtime
    cost ~35us measured).
"""

import os
import numpy as np
from contextlib import ExitStack

import concourse.bass as bass
import concourse.tile as tile
from concourse import mybir, bacc
from concourse.bass_utils import run_bass_kernel_spmd

F32 = mybir.dt.float32
F16 = mybir.dt.float16
AL = mybir.AluOpType
AF = mybir.ActivationFunctionType

P = 128
NB = 512                              # psum block width (out-dims per block)
DIMS = [2048, 2048, 2048, 2048, 1000]
PD = [2048, 2048, 2048, 2048, 1024]  # padded dims
KT = [d // P for d in PD]            # [16, 16, 16, 16, 8] k-tiles per dim
BATCH = 1024
N_CORES = 8
BPC = BATCH // N_CORES               # 128 batch rows per core
N_RELAX = int(os.environ.get("KERNEL_N_RELAX", "25"))
LR = 0.3
INV_LR = float(np.float32(1.0 / 0.3))

MM_DT = F16
MM_NP = np.float16

# wpack rows (each [P, 2048] fp16): panel-quad sets for streamed weights,
# k-slab sets for resident ones.
WOFF = {"w0s": 0, "w1f": 16, "w1b": 32, "w2f": 48, "w2b": 64, "w3b": 80,
        "w3fp": 88}
WROWS = 96


def _panel_quads(W, Kp, Mp, scale):
    """Streamed layout: row[nb*4+q][p, qi*512+j] = scale*W[(q*4+qi)*128+p, nb*512+j]."""
    K, M = W.shape
    Wp = np.zeros((Kp, Mp), np.float32)
    Wp[:K, :M] = W * scale
    assert Kp == 2048 and Mp % NB == 0
    t = Wp.reshape(4, 4, P, Mp // NB, NB)            # [q, qi, p, nb, j]
    out = t.transpose(3, 0, 2, 1, 4).reshape(Mp // NB * 4, P, 2048)
    return np.ascontiguousarray(out).astype(MM_NP)


def _kslabs(W, Kp, Mp, scale):
    """Resident layout: row[k][p, j] = scale*W[k*128+p, j]."""
    K, M = W.shape
    Wp = np.zeros((Kp, Mp), np.float32)
    Wp[:K, :M] = W * scale
    return (Wp.reshape(Kp // P, P, Mp) * 1.0).astype(MM_NP)


def build_nc():
    nc = bacc.Bacc(None, target_bir_lowering=False, debug=False)

    d_w = nc.declare_dram_parameter("wpack", [WROWS, P, 2048], F16, isOutput=False)
    # mpack rows: 0=x16T, 1=cx16T, 2=[i10|i07|...], 3=bias rows:
    #   partition 0: 0.3*b1 (2048), p1: 0.3*b2, p2: 0.3*b3, p3: 0.3*b4 (1024)
    d_m = nc.declare_dram_parameter("mpack", [4, P, 2048], F16, isOutput=False)
    d_out = nc.declare_dram_parameter("out", [P, PD[4]], F16, isOutput=True)

    with tile.TileContext(nc) as tc, ExitStack() as ctx:
        st = ctx.enter_context(tc.tile_pool(name="state", bufs=1))
        wp = ctx.enter_context(tc.tile_pool(name="wslab", bufs=5))
        psw = ctx.enter_context(tc.tile_pool(name="psw", bufs=4, space="PSUM"))
        ptp = ctx.enter_context(tc.tile_pool(name="ptp", bufs=4, space="PSUM"))
        bmp = ctx.enter_context(tc.tile_pool(name="bmp", bufs=4))
        tp = ctx.enter_context(tc.tile_pool(name="tmp", bufs=2))

        # persistent fp16 states, dim-major [dim-in-tile, m-tile*P + batch]
        sT = {}
        for l in range(1, 5):
            sT[l] = st.tile([P, PD[l]], F16, tag=f"sT{l}", name=f"sT{l}")
        add1 = st.tile([P, PD[1]], F16, tag="add1")   # 0.3*(rho(x)@W0+b1), dim-major
        ii = st.tile([P, 2 * P], F16, tag="ii")       # [i10 | i07]
        ones = st.tile([1, P], F16, tag="ones")
        brow = {}
        for l, part in ((1, 0), (2, 1), (3, 2), (4, 3)):
            brow[l] = st.tile([1, PD[l]], F16, tag=f"b{l}s", name=f"b{l}s")
            nc.sync.dma_start(brow[l][:], d_m[3, part:part + 1, :PD[l]])
        nc.sync.dma_start(ii[:], d_m[2, :, : 2 * P])
        i10 = ii[:, 0:P]
        i07 = ii[:, P:2 * P]
        nc.vector.memset(ones[:], 1.0)
        x16 = wp.tile([P, PD[0]], F16, tag="slab", name="x16t")
        cx16 = wp.tile([P, PD[0]], F16, tag="slab", name="cx16t")
        nc.sync.dma_start(x16[:], d_m[0])
        nc.sync.dma_start(cx16[:], d_m[1])

        # resident weights
        w2b_res = st.tile([P, KT[2] * PD[3]], MM_DT, tag="w2b_res")
        w3b_res = st.tile([P, KT[4] * PD[3]], MM_DT, tag="w3b_res")
        w3f_res = st.tile([P, 8 * 2048], MM_DT, tag="w3f_res")
        w2f_res = st.tile([P, 4 * 2048], MM_DT, tag="w2f_res")

        def fwd_quads(ps, wname, s_in, first):
            """Streamed panel quads for one block: 16 MMs of N=512."""
            def emit(nb):
                for q in range(4):
                    wq = wp.tile([P, 2048], MM_DT, tag="slab")
                    nc.sync.dma_start(wq[:], d_w[WOFF[wname] + nb * 4 + q])
                    for qi in range(4):
                        k = q * 4 + qi
                        nc.tensor.matmul(
                            ps[:], s_in[:, bass.ts(k, P)],
                            wq[:, bass.ts(qi, NB)],
                            start=(first and k == 0), stop=False,
                        )
            return emit

        def res_mms(ps, res, kt, s_in, nb, first, row_w=2048):
            for k in range(kt):
                nc.tensor.matmul(
                    ps[:], s_in[:, bass.ts(k, P)],
                    res[:, k * row_w + nb * NB : k * row_w + nb * NB + NB],
                    start=(first and k == 0), stop=False,
                )

        def extras(ps, lo, nb, with_add1, stop_on_last):
            """bias rank-1 (already in brow, 0.3-scaled) + add1 + 0.7*s."""
            nc.tensor.matmul(
                ps[:], ones[:], brow[lo][:, nb * NB : nb * NB + NB],
                start=False, stop=False,
            )
            if with_add1:
                for c in range(4):
                    nc.tensor.matmul(
                        ps[:, bass.ts(c, P)], add1[:, bass.ts(nb * 4 + c, P)],
                        i10, start=False, stop=False,
                    )
            for c in range(4):
                nc.tensor.matmul(
                    ps[:, bass.ts(c, P)], sT[lo][:, bass.ts(nb * 4 + c, P)],
                    i07, start=False, stop=(stop_on_last and c == 3),
                )

        def transpose_out(bm, dst, nb):
            for c in range(4):
                pt = ptp.tile([P, P], F16, tag="pt")
                nc.tensor.transpose(pt[:], bm[:, bass.ts(c, P)], i10)
                nc.scalar.activation(
                    dst[:, bass.ts(nb * 4 + c, P)], pt[:], AF.Copy
                )

        # ---- init pass (scaled weights; rescale by 1/0.3 before clip) ----
        # layer 1 + add1 share one pass over w0s panels
        pend = []
        for nb in range(4):
            ps_i = psw.tile([P, NB], F32, tag="ps")
            ps_c = psw.tile([P, NB], F32, tag="ps")
            for q in range(4):
                wq = wp.tile([P, 2048], MM_DT, tag="slab")
                nc.sync.dma_start(wq[:], d_w[WOFF["w0s"] + nb * 4 + q])
                for qi in range(4):
                    k = q * 4 + qi
                    nc.tensor.matmul(ps_i[:], x16[:, bass.ts(k, P)],
                                     wq[:, bass.ts(qi, NB)],
                                     start=(k == 0), stop=False)
                    nc.tensor.matmul(ps_c[:], cx16[:, bass.ts(k, P)],
                                     wq[:, bass.ts(qi, NB)],
                                     start=(k == 0), stop=False)
            nc.tensor.matmul(ps_i[:], ones[:], brow[1][:, nb * NB : nb * NB + NB],
                             start=False, stop=True)
            nc.tensor.matmul(ps_c[:], ones[:], brow[1][:, nb * NB : nb * NB + NB],
                             start=False, stop=True)
            # s1 = clip(ps_i/0.3), add1 = ps_c
            t = tp.tile([P, NB], F32, tag="t")
            nc.vector.tensor_scalar(t[:], ps_i[:], INV_LR, 0.0, AL.mult, AL.add)
            bm_i = bmp.tile([P, NB], F16, tag="bm")
            nc.vector.tensor_scalar(bm_i[:], t[:], 0.0, 1.0, AL.max, AL.min)
            bm_c = bmp.tile([P, NB], F16, tag="bm")
            nc.vector.tensor_copy(bm_c[:], ps_c[:])
            for prev in pend:
                transpose_out(*prev)
            pend = [(bm_i, sT[1], nb), (bm_c, add1, nb)]
        for prev in pend:
            transpose_out(*prev)

        def init_layer(lo, s_in, emit_mms, blocks):
            pend = None
            for nb in range(blocks):
                ps = psw.tile([P, NB], F32, tag="ps")
                emit_mms(ps, nb)
                nc.tensor.matmul(
                    ps[:], ones[:], brow[lo][:, nb * NB : nb * NB + NB],
                    start=False, stop=True,
                )
                t = tp.tile([P, NB], F32, tag="t")
                nc.vector.tensor_scalar(t[:], ps[:], INV_LR, 0.0, AL.mult, AL.add)
                bm = bmp.tile([P, NB], F16, tag="bm")
                nc.vector.tensor_scalar(bm[:], t[:], 0.0, 1.0, AL.max, AL.min)
                if pend is not None:
                    transpose_out(*pend)
                pend = (bm, sT[lo], nb)
            transpose_out(*pend)

        def l2_init(ps, nb):
            fwd_quads(ps, "w1f", sT[1], True)(nb)
        init_layer(2, sT[1], l2_init, 4)

        for m in range(4):
            nc.sync.dma_start(
                w2f_res[:, m * 2048 : (m + 1) * 2048], d_w[WOFF["w2f"] + m]
            )

        def l3_init(ps, nb):
            if nb == 0:
                for q in range(4):
                    for qi in range(4):
                        k = q * 4 + qi
                        nc.tensor.matmul(
                            ps[:], sT[2][:, bass.ts(k, P)],
                            w2f_res[:, q * 2048 + qi * NB : q * 2048 + qi * NB + NB],
                            start=(k == 0), stop=False,
                        )
            else:
                fwd_quads(ps, "w2f", sT[2], True)(nb)
        init_layer(3, sT[2], l3_init, 4)

        for m in range(8):
            nc.sync.dma_start(
                w3f_res[:, m * 2048 : (m + 1) * 2048], d_w[WOFF["w3fp"] + m]
            )

        def l4_init(ps, nb):
            for k in range(KT[3]):
                nc.tensor.matmul(
                    ps[:], sT[3][:, bass.ts(k, P)],
                    w3f_res[:, (k // 2) * 2048 + (k % 2) * PD[4] + nb * NB :
                            (k // 2) * 2048 + (k % 2) * PD[4] + nb * NB + NB],
                    start=(k == 0), stop=False,
                )
        init_layer(4, sT[3], l4_init, 2)

        for m in range(KT[2]):
            nc.sync.dma_start(
                w2b_res[:, m * PD[3] : (m + 1) * PD[3]], d_w[WOFF["w2b"] + m]
            )
        for m in range(KT[4]):
            nc.sync.dma_start(
                w3b_res[:, m * PD[3] : (m + 1) * PD[3]], d_w[WOFF["w3b"] + m]
            )

        # ---- relaxation sweeps ----
        def sweep_layer(lo, blocks, emit_mms, with_add1):
            pend = None
            for nb in range(blocks):
                ps = psw.tile([P, NB], F32, tag="ps")
                emit_mms(ps, nb)
                extras(ps, lo, nb, with_add1, True)
                bm = bmp.tile([P, NB], F16, tag="bm")
                nc.vector.tensor_scalar(bm[:], ps[:], 0.0, 1.0, AL.max, AL.min)
                if pend is not None:
                    transpose_out(*pend)
                pend = (bm, sT[lo], nb)
            transpose_out(*pend)

        for _ in range(N_RELAX):
            # layer 1: add1 + 0.7*s1 + bwd (streamed w1b panels, contract s2)
            sweep_layer(
                1, 4, lambda ps, nb: fwd_quads(ps, "w1b", sT[2], True)(nb), True
            )

            # layer 2: bias + 0.7*s2 + fwd (streamed w1f) + bwd (resident w2b)
            def l2_mms(ps, nb):
                fwd_quads(ps, "w1f", sT[1], True)(nb)
                res_mms(ps, w2b_res, KT[3], sT[3], nb, False)
            sweep_layer(2, 4, l2_mms, False)

            # layer 3: bias + 0.7*s3 + fwd (res panel + streamed w2f) + bwd (res w3b)
            def l3_mms(ps, nb):
                if nb == 0:
                    for q in range(4):
                        for qi in range(4):
                            k = q * 4 + qi
                            nc.tensor.matmul(
                                ps[:], sT[2][:, bass.ts(k, P)],
                                w2f_res[:, q * 2048 + qi * NB :
                                        q * 2048 + qi * NB + NB],
                                start=(k == 0), stop=False,
                            )
                else:
                    fwd_quads(ps, "w2f", sT[2], True)(nb)
                res_mms(ps, w3b_res, KT[4], sT[4], nb, False)
            sweep_layer(3, 4, l3_mms, False)

            # layer 4: bias + 0.7*s4 + fwd (resident packed w3f)
            def l4_mms(ps, nb):
                for k in range(KT[3]):
                    nc.tensor.matmul(
                        ps[:], sT[3][:, bass.ts(k, P)],
                        w3f_res[:, (k // 2) * 2048 + (k % 2) * PD[4] + nb * NB :
                                (k // 2) * 2048 + (k % 2) * PD[4] + nb * NB + NB],
                        start=(k == 0), stop=False,
                    )
            sweep_layer(4, 2, l4_mms, False)

        nc.sync.dma_start(d_out[:], sT[4][:])

    nc.compile()
    return nc


def _prep_inputs(x, W0, W1, W2, W3, b1, b2, b3, b4):
    """Host-side data prep shared by all cores (weights) + per-core x."""
    W1T = np.ascontiguousarray(np.asarray(W1, np.float32).T)
    wpack = np.zeros((WROWS, P, 2048), MM_NP)
    wpack[WOFF["w0s"]:WOFF["w0s"] + 16] = _panel_quads(W0, PD[0], PD[1], LR)
    wpack[WOFF["w1f"]:WOFF["w1f"] + 16] = _panel_quads(W1, PD[1], PD[2], LR)
    wpack[WOFF["w1b"]:WOFF["w1b"] + 16] = _panel_quads(W1T, PD[2], PD[1], LR)
    wpack[WOFF["w2f"]:WOFF["w2f"] + 16] = _panel_quads(W2, PD[2], PD[3], LR)
    W2T = np.ascontiguousarray(np.asarray(W2, np.float32).T)
    wpack[WOFF["w2b"]:WOFF["w2b"] + 16] = _kslabs(W2T, PD[3], PD[2], LR)
    W3T = np.ascontiguousarray(np.asarray(W3, np.float32).T)
    wpack[WOFF["w3b"]:WOFF["w3b"] + 8] = _kslabs(W3T, PD[4], PD[3], LR)
    w3k = _kslabs(W3, PD[3], PD[4], LR)              # [16, P, 1024]
    wpack[WOFF["w3fp"]:WOFF["w3fp"] + 8] = w3k.reshape(8, 2, P, PD[4]).transpose(
        0, 2, 1, 3).reshape(8, P, 2048)

    mrow3 = np.zeros((P, 2048), MM_NP)
    for i, (b, pd) in enumerate(zip([b1, b2, b3, b4], PD[1:])):
        bf = np.asarray(b, np.float32)
        mrow3[i, :pd] = (np.pad(bf, (0, pd - len(bf))) * LR).astype(MM_NP)
    mrow2 = np.zeros((P, 2048), MM_NP)
    mrow2[:, :P] = np.eye(P, dtype=MM_NP)
    mrow2[:, P:2 * P] = (0.7 * np.eye(P)).astype(MM_NP)

    in_maps = []
    for c in range(N_CORES):
        xs = np.asarray(x[c * BPC : (c + 1) * BPC], dtype=np.float32)
        # xT[p, k*P+j] = xs[j, k*P+p]
        xT = np.ascontiguousarray(
            xs.reshape(BPC, PD[0] // P, P).transpose(2, 1, 0)
        ).reshape(P, PD[0])
        mpack = np.stack([
            xT.astype(MM_NP),
            np.clip(xT, 0.0, 1.0).astype(MM_NP),
            mrow2,
            mrow3,
        ])
        in_maps.append({"wpack": wpack, "mpack": mpack})
    return in_maps


_NC_CACHE = None


def _get_nc():
    global _NC_CACHE
    if _NC_CACHE is None:
        _NC_CACHE = build_nc()
    return _NC_CACHE


def decode_output(out_map):
    """Decode {'out': [N_CORES*P, PD[4]]} to [BATCH, 1000] float32."""
    o_all = out_map["out"]
    outs = []
    for c in range(N_CORES):
        o = np.asarray(o_all[c * P : (c + 1) * P])  # [P, PD[4]] = [128, 1024]
        # decode: o[p, k*P+j] = s4T[k*P+p, j] = s4[batch j, dim k*P+p]
        s4 = o.reshape(P, PD[4] // P, P).transpose(2, 1, 0).reshape(BPC, PD[4])
        outs.append(s4[:, : DIMS[4]])
    return np.concatenate(outs, axis=0).astype(np.float32)


def run(inputs, trace=False):
    nc = _get_nc()
    in_maps = _prep_inputs(**inputs)
    res = run_bass_kernel_spmd(nc, in_maps, list(range(N_CORES)), trace=trace)
    full = np.concatenate([res.results[c]["out"] for c in range(N_CORES)], axis=0)
    return decode_output({"out": full}), res


def kernel(**inputs):
    out, _ = run(inputs, trace=False)
    return out
